# revision 5
# baseline (speedup 1.0000x reference)
"""MGNN (gnn_message_passing) Trainium2 kernel.

Strategy (8 NeuronCores, destination-sharded, no collectives):
  - Each core owns N/8 = 6250 destination nodes. Host partitions the edge
    lists by destination row, sorts by (local) destination, and pads edge
    chunks so all 8 cores run an identical SPMD program.
  - Aggregation identity: agg_i = segsum(val * (h @ W_i^T)[col])
                                = segsum(val * h[col]) @ W_i^T
    so the gather table is h itself for all 3 metapaths; the per-metapath
    weight matmul is applied after aggregation (on [D, n] tiles).
  - On device (feature-major layout [D=128 partitions, nodes on free dim]):
      * batched indirect-DMA gathers of h rows (128 rows/chunk, 32 chunks
        per DMA instruction); out-of-range pad indices are skipped via
        bounds_check (no HBM traffic for padding).
      * segment-sum via one-hot matmul: S[e, d] = val_e * (iota[d]==doff_e),
        PSUM accumulation per 32-destination window.
      * FiLM folded into weights: gamma ∈ {g0, g1} by node_type, handled by
        sorting each core's nodes by type (host) and using two pre-scaled
        weight matrices W0 = diag(g0) W, W1 = diag(g1) W. seq_fts residual
        is matmul-accumulated into the same PSUM tile.
      * PReLU(u) = max(u, a*u) via two scalar-engine affine ops + vector max.
      * Semantics attention: tanh/score matmuls in feature-major form,
        softmax computed node-major after an SBUF reshape DMA.
  - Output is written feature-major [128, NCOL]; host transposes, strips
    padding, undoes the type-sort permutation and concatenates shards.
"""

import math
import os

import numpy as np


def _ensure_path():
    try:
        import concourse  # noqa: F401
    except ImportError:
        import sys

        for p in ("/opt/trn_rl_repo", "/root/.axon_site/_ro/trn_rl_repo"):
            if os.path.isdir(p) and p not in sys.path:
                sys.path.insert(0, p)


# ---------------------------------------------------------------------------
# configuration
# ---------------------------------------------------------------------------

N_CORES = 8
D = 128          # hidden dim (= partition count)
CHUNK = 128      # edges per matmul chunk (contraction dim)
WIN = 64         # destinations per one-hot window (S width)
KG = 16          # chunks per dma_gather instruction
BANK = 512      # psum bank width (f32 elems) = 16 windows
PAD_COL = 1 << 28  # out-of-bounds gather index for pad edges (skipped)

F32 = np.float32
I32 = np.int32


# ---------------------------------------------------------------------------
# host-side planning
# ---------------------------------------------------------------------------

def _round_up(x, m):
    return (x + m - 1) // m * m


def _plan(h, edge_rows, edge_cols, edge_vals, node_type):
    """Shard by destination, type-sort each shard, build padded chunk plan.

    Chunks are segregated by source half (dma_gather indices are int16, so
    the gather table is split at NLO = N//2). Returns (cfg, per_core).
    """
    N = h.shape[0]
    P = edge_rows.shape[0]
    npc = N // N_CORES
    assert npc * N_CORES == N
    NLO = N // 2
    assert NLO <= 32768 and (N - NLO) <= 32768

    # --- per-core destination shards, sorted by node_type (stable) ---
    shards = []
    for c in range(N_CORES):
        own = slice(c * npc, (c + 1) * npc)
        t = node_type[own]
        perm = np.argsort(t, kind="stable")  # sorted-rank -> original local id
        n0 = int((t == 0).sum())
        shards.append({"perm": perm, "n0": n0})

    max_n0 = max(s["n0"] for s in shards)
    max_n1 = max(npc - s["n0"] for s in shards)
    B0 = _round_up(max(max_n0, 1), BANK)
    NCOL = B0 + _round_up(max(max_n1, 1), BANK)
    NBANK = NCOL // BANK
    NWIN = NCOL // WIN

    # padded-column map per core: local node id -> column
    for s in shards:
        inv = np.empty(npc, dtype=np.int64)
        inv[s["perm"]] = np.arange(npc)  # original local id -> sorted rank
        col = np.where(inv < s["n0"], inv, B0 + (inv - s["n0"]))
        s["colmap"] = col

    # --- edge bucketing by (core, metapath, half, window) ---
    edge_data = [[None] * P for _ in range(N_CORES)]
    hist = np.zeros((2, N_CORES, P, NWIN), dtype=np.int64)
    for c in range(N_CORES):
        base = c * npc
        for m in range(P):
            er = edge_rows[m]
            mask = (er >= base) & (er < base + npc)
            dl = shards[c]["colmap"][er[mask] - base]
            cs = edge_cols[m][mask].astype(np.int64)
            vs = edge_vals[m][mask].astype(F32)
            half = (cs >= NLO).astype(np.int64)
            # sort by (window, half) so each (w, half) group is contiguous
            key = (dl // WIN) * 2 + half
            order = np.argsort(key, kind="stable")
            dl = dl[order]
            cs = cs[order]
            vs = vs[order]
            half = half[order]
            w = dl // WIN
            for hf in range(2):
                hist[hf, c, m] += np.bincount(w[half == hf], minlength=NWIN)
            edge_data[c][m] = (dl, cs, vs, w, half)

    cl = np.maximum(1, -(-hist[0].max(axis=0) // CHUNK))   # [P, NWIN] lo
    ch = -(-hist[1].max(axis=0) // CHUNK)                  # [P, NWIN] hi
    counts2 = np.stack([cl, ch])                           # [2, P, NWIN]
    nch = [int(cl.sum()), int(ch.sum())]
    nch_pad = [_round_up(x, KG) for x in nch]

    # chunk slot base per (half, m, w) within its stream
    base_slot = np.zeros((2, P, NWIN), dtype=np.int64)
    for hf in range(2):
        flat = counts2[hf].reshape(-1)
        base_slot[hf].reshape(-1)[1:] = np.cumsum(flat)[:-1]

    per_core = []
    for c in range(N_CORES):
        streams = []
        for hf in range(2):
            nitems = nch_pad[hf] * CHUNK
            streams.append({
                "idx": np.full(nitems, -1, dtype=np.int64),
                "doff": np.zeros(nitems, dtype=F32),
                "val": np.zeros(nitems, dtype=F32),
            })
            # in-plan chunks: pad items default to row 0 / val 0
            ninplan = int(counts2[hf].sum()) * CHUNK
            streams[hf]["idx"][:ninplan] = 0
        for m in range(P):
            dl, cs, vs, w, half = edge_data[c][m]
            for hf in range(2):
                sel = half == hf
                wm_ = w[sel]
                starts = np.searchsorted(wm_, np.arange(NWIN))
                rank = np.arange(len(wm_)) - starts[wm_]
                slot = base_slot[hf, m, wm_] * CHUNK + rank
                st = streams[hf]
                st["idx"][slot] = cs[sel] - hf * NLO
                st["doff"][slot] = (dl[sel] - wm_ * WIN).astype(F32)
                st["val"][slot] = vs[sel]
        pc = {"perm": shards[c]["perm"], "n0": shards[c]["n0"]}
        for hf, tag in ((0, "L"), (1, "H")):
            st = streams[hf]
            # idx items wrapped in 16 partitions, replicated to 128
            iw = np.ascontiguousarray(
                st["idx"].reshape(-1, 16).T).astype(np.int16)   # [16, items/16]
            pc["idx" + tag] = np.tile(iw, (8, 1))               # [128, items/16]
            pc["doff" + tag] = np.ascontiguousarray(
                st["doff"].reshape(-1, CHUNK).T)                # [128, nch_pad]
            pc["val" + tag] = np.ascontiguousarray(
                st["val"].reshape(-1, CHUNK).T)
        per_core.append(pc)

    # per-gather-batch valid index counts (same for all cores by construction)
    nreg = []
    for hf in range(2):
        ninplan = int(counts2[hf].sum())
        nb = nch_pad[hf] // KG
        r = []
        for g in range(nb):
            lo_c = g * KG
            r.append(max(0, min(ninplan - lo_c, KG)) * CHUNK)
        nreg.append(r)

    cfg = {
        "N": N,
        "NLO": NLO,
        "P": P,
        "npc": npc,
        "B0": B0,
        "NCOL": NCOL,
        "NBANK": NBANK,
        "NWIN": NWIN,
        "counts2": counts2,
        "nch_pad": nch_pad,
        "nreg": nreg,
    }
    return cfg, per_core


def _pack_weights(cfg, W_fc, prelu_a, Wg, bg, Wb, bb, film_bias,
                  att_W1, att_b1, att_w2):
    """Pack small weights into two dense blobs (replicated to every core)."""
    P = cfg["P"]
    # wmats: per meta [W0T, W1T, WfcT], then att_W1T  -> [128, (3P+1)*128]
    blocks = []
    for m in range(P):
        g0 = (Wg[m][:, 0] + bg[m]).astype(F32)  # [D]
        g1 = (Wg[m][:, 1] + bg[m]).astype(F32)
        WT = W_fc[m].T.astype(F32)              # [fi, fo]
        blocks += [WT * g0[None, :], WT * g1[None, :], WT]
    blocks.append(att_W1.T.astype(F32))          # lhsT[d, hid]
    wmats = np.ascontiguousarray(np.concatenate(blocks, axis=1))

    # cvec [128, WIN+16]: iota window, b1, w2,
    # per-meta (bfb0, bfb1, a*bfb0, a*bfb1)
    cvec = np.zeros((D, WIN + 16), dtype=F32)
    cvec[:, :WIN] = np.arange(WIN, dtype=F32)[None, :]
    cvec[:, WIN] = att_b1.astype(F32)
    cvec[:, WIN + 1] = att_w2.astype(F32)
    for m in range(P):
        a = float(prelu_a[m])
        bfb0 = (Wb[m][:, 0] + bb[m] + film_bias[m]).astype(F32)
        bfb1 = (Wb[m][:, 1] + bb[m] + film_bias[m]).astype(F32)
        cvec[:, WIN + 2 + 4 * m] = bfb0
        cvec[:, WIN + 3 + 4 * m] = bfb1
        cvec[:, WIN + 4 + 4 * m] = a * bfb0
        cvec[:, WIN + 5 + 4 * m] = a * bfb1
    return wmats, cvec


# ---------------------------------------------------------------------------
# device program
# ---------------------------------------------------------------------------

def _build_program(cfg, alphas, stage=99):
    _ensure_path()
    import concourse.bass as bass  # noqa: F401
    import concourse.tile as tile
    from concourse import bacc, mybir

    P = cfg["P"]
    NCOL = cfg["NCOL"]
    NBANK = cfg["NBANK"]
    counts2 = cfg["counts2"]
    nch_pad = cfg["nch_pad"]
    nreg = cfg["nreg"]
    N = cfg["N"]
    NLO = cfg["NLO"]
    dt = mybir.dt
    f32 = dt.float32

    nc = bacc.Bacc(
        "TRN2",
        target_bir_lowering=False,
        debug=False,
        enable_asserts=False,
        num_devices=N_CORES,
    )

    h_tab = nc.dram_tensor("h_tab", [N, D], f32, kind="ExternalInput").ap()
    hT = nc.dram_tensor("hT", [D, NCOL], f32, kind="ExternalInput").ap()
    idxd = {}
    doffd = {}
    vald = {}
    for hf, tag in ((0, "L"), (1, "H")):
        ni = max(nch_pad[hf] * CHUNK // 16, 1)
        idxd[hf] = nc.dram_tensor(f"idx{tag}", [CHUNK, ni], dt.int16,
                                  kind="ExternalInput").ap()
        nch1 = max(nch_pad[hf], 1)
        doffd[hf] = nc.dram_tensor(f"doff{tag}", [CHUNK, nch1], f32,
                                   kind="ExternalInput").ap()
        vald[hf] = nc.dram_tensor(f"val{tag}", [CHUNK, nch1], f32,
                                  kind="ExternalInput").ap()
    wmatsd = nc.dram_tensor("wmats", [D, (3 * P + 1) * D], f32,
                            kind="ExternalInput").ap()
    cvecd = nc.dram_tensor("cvec", [D, WIN + 16], f32, kind="ExternalInput").ap()
    outd = nc.dram_tensor("outT", [D, NCOL], f32, kind="ExternalOutput").ap()
    zspill = nc.dram_tensor("z_spill", [P, D, NCOL], f32, kind="Internal").ap()

    half_tab = {0: h_tab[0:NLO, :], 1: h_tab[NLO:N, :]}

    with tile.TileContext(nc) as tc, tc.tile_pool(name="const", bufs=1) as cpool, \
            tc.tile_pool(name="gpool", bufs=2) as gpool, \
            tc.tile_pool(name="spool", bufs=2) as spool, \
            tc.tile_pool(name="mpool", bufs=2) as mpool, \
            tc.tile_pool(name="work", bufs=2) as work, \
            tc.tile_pool(name="ps_agg", bufs=3, space="PSUM") as ps_agg, \
            tc.tile_pool(name="ps_misc", bufs=2, space="PSUM") as ps_misc, \
            tc.tile_pool(name="ps_attn", bufs=2, space="PSUM") as ps_attn:

        # ---- constants / resident inputs ----
        hT_t = cpool.tile([D, NCOL], f32, tag="hT", name="hT")
        nc.sync.dma_start(out=hT_t[:], in_=hT)
        wm_t = cpool.tile([D, (3 * P + 1) * D], f32, tag="wm", name="wm")
        nc.sync.dma_start(out=wm_t[:], in_=wmatsd)
        cv_t = cpool.tile([D, WIN + 16], f32, tag="cv", name="cv")
        nc.sync.dma_start(out=cv_t[:], in_=cvecd)
        ones_t = cpool.tile([65, D], f32, tag="ones", name="ones")
        nc.vector.memset(ones_t[:], 1.0)

        def wmat(i):  # [128,128] lhsT block i
            return wm_t[:, i * D:(i + 1) * D]

        attW1T = wmat(3 * P)
        iota = cv_t[:, 0:WIN]
        b1c = cv_t[:, WIN:WIN + 1]
        w2c = cv_t[:, WIN + 1:WIN + 2]

        # partitions 0/32/64 hold s_m then beta_m (WAR-serialized)
        rows_t = cpool.tile([65, NCOL], f32, tag="rows", name="rows")

        # ---- gather + S build, two half streams ----
        # Pre-zero gather-pool slots: trailing pad indices (-1) are skipped
        # by dma_gather (no write); uninitialized SBUF may hold NaN which
        # S=0 would not mask (NaN*0=NaN in the matmul).
        for tg in ("gL", "gH"):
            for _ in range(2):
                gw = gpool.tile([CHUNK, KG * D], f32, tag=tg, name=tg)
                nc.vector.memset(gw[:], 0.0)

        gtiles = {}
        stiles = {}

        def ensure_batch(hf, g):
            if (hf, g) in gtiles:
                return
            tg = "gL" if hf == 0 else "gH"
            ix = mpool.tile([CHUNK, KG * CHUNK // 16], dt.int16,
                            tag="ix" + tg, name="ix" + tg)
            nc.sync.dma_start(
                out=ix[:],
                in_=idxd[hf][:, g * (KG * CHUNK // 16):(g + 1) * (KG * CHUNK // 16)])
            mdoff = mpool.tile([CHUNK, KG], f32, tag="md" + tg, name="md" + tg)
            nc.sync.dma_start(out=mdoff[:], in_=doffd[hf][:, g * KG:(g + 1) * KG])
            mval = mpool.tile([CHUNK, KG], f32, tag="mv" + tg, name="mv" + tg)
            nc.sync.dma_start(out=mval[:], in_=vald[hf][:, g * KG:(g + 1) * KG])
            gt = gpool.tile([CHUNK, KG * D], f32, tag=tg, name=tg)
            nc.gpsimd.dma_gather(
                out_ap=gt[:].rearrange("p (k d) -> p k d", k=KG),
                in_ap=half_tab[hf],
                idxs_ap=ix[:],
                num_idxs=KG * CHUNK,
                num_idxs_reg=int(nreg[hf][g]),
                elem_size=D,
                single_packet=False,
            )
            eq = spool.tile([CHUNK, KG * WIN], f32, tag="eq", name="eq",
                            bufs=1)
            st = spool.tile([CHUNK, KG * WIN], f32, tag="s" + tg,
                            name="s" + tg)
            nc.vector.tensor_tensor(
                out=eq[:],
                in0=iota.unsqueeze(1).to_broadcast([CHUNK, KG, WIN]),
                in1=mdoff[:].unsqueeze(2).to_broadcast([CHUNK, KG, WIN]),
                op=mybir.AluOpType.is_equal,
            )
            nc.vector.tensor_tensor(
                out=st[:],
                in0=eq[:],
                in1=mval[:].unsqueeze(2).to_broadcast([CHUNK, KG, WIN]),
                op=mybir.AluOpType.mult,
            )
            gtiles[(hf, g)] = gt
            stiles[(hf, g)] = st

        chunk_c = [0, 0]
        WPB = BANK // WIN  # windows per bank

        for m in range(P):
            for b in range(NBANK):
                agg = ps_agg.tile([D, BANK], f32, space="PSUM", tag="agg", name="agg")
                for wl in range(WPB):
                    w = b * WPB + wl
                    tot = int(counts2[0, m, w]) + int(counts2[1, m, w])
                    j = 0
                    for hf in range(2):
                        for _ in range(int(counts2[hf, m, w])):
                            g, cl = divmod(chunk_c[hf], KG)
                            ensure_batch(hf, g)
                            nc.tensor.matmul(
                                out=agg[:, wl * WIN:(wl + 1) * WIN],
                                lhsT=gtiles[(hf, g)][:, cl * D:(cl + 1) * D],
                                rhs=stiles[(hf, g)][:, cl * WIN:(cl + 1) * WIN],
                                start=(j == 0),
                                stop=(j == tot - 1),
                            )
                            chunk_c[hf] += 1
                            j += 1
                # evacuate A@h bank to SBUF (scalar engine copy)
                aggh = work.tile([D, BANK], f32, tag="aggh", name="aggh")
                nc.scalar.copy(out=aggh[:], in_=agg[:])
                if stage < 2:
                    nc.sync.dma_start(out=outd[:, slice(b * BANK, (b + 1) * BANK)],
                                      in_=aggh[:])
                    continue
                # z_pre^T = W_t . aggh + W . hT   (accumulated in PSUM)
                fps = ps_misc.tile([D, BANK], f32, space="PSUM", tag="fps", name="fps")
                wsel = 3 * m + (0 if b < cfg["B0"] // BANK else 1)
                csl = slice(b * BANK, (b + 1) * BANK)
                nc.tensor.matmul(out=fps[:], lhsT=wmat(wsel), rhs=aggh[:],
                                 start=True, stop=False)
                nc.tensor.matmul(out=fps[:], lhsT=wmat(3 * m + 2),
                                 rhs=hT_t[:, csl], start=False, stop=True)
                # PReLU(u + bfb) = max(u + bfb, a*u + a*bfb)
                ty = 0 if b < cfg["B0"] // BANK else 1
                bfb = cv_t[:, WIN + 2 + 4 * m + ty:WIN + 3 + 4 * m + ty]
                abfb = cv_t[:, WIN + 4 + 4 * m + ty:WIN + 5 + 4 * m + ty]
                t0 = work.tile([D, BANK], f32, tag="t0", name="t0")
                t1 = work.tile([D, BANK], f32, tag="t1", name="t1")
                nc.scalar.activation(t0[:], fps[:],
                                     mybir.ActivationFunctionType.Identity,
                                     bias=bfb, scale=1.0)
                nc.scalar.activation(t1[:], fps[:],
                                     mybir.ActivationFunctionType.Identity,
                                     bias=abfb, scale=float(alphas[m]))
                zb = work.tile([D, BANK], f32, tag="zb", name="zb")
                nc.vector.tensor_tensor(out=zb[:], in0=t0[:],
                                        in1=t1[:], op=mybir.AluOpType.max)
                nc.sync.dma_start(out=zspill[m, :, csl], in_=zb[:])
                # attention scores for this bank
                if stage < 3:
                    continue
                aps = ps_attn.tile([D, BANK], f32, space="PSUM", tag="at", name="at")
                nc.tensor.matmul(out=aps[:], lhsT=attW1T, rhs=zb[:],
                                 start=True, stop=True)
                th = work.tile([D, BANK], f32, tag="tanh", name="tanh")
                nc.scalar.activation(th[:], aps[:],
                                     mybir.ActivationFunctionType.Tanh,
                                     bias=b1c, scale=1.0)
                sps = ps_attn.tile([1, BANK], f32, space="PSUM", tag="at", name="at")
                nc.tensor.matmul(out=sps[:], lhsT=w2c, rhs=th[:],
                                 start=True, stop=True)
                nc.scalar.copy(out=rows_t[32 * m:32 * m + 1, csl], in_=sps[:])

        assert chunk_c[0] == int(counts2[0].sum())
        assert chunk_c[1] == int(counts2[1].sum())

        # ---- softmax over metapaths (node-major [128, NCOL/128]) ----
        if stage < 4:
            if stage >= 2:
                for b in range(NBANK):
                    csl = slice(b * BANK, (b + 1) * BANK)
                    zl0 = work.tile([D, BANK], f32, tag="zl", name="zl", bufs=4)
                    nc.sync.dma_start(out=zl0[:], in_=zspill[0, :, csl])
                    nc.sync.dma_start(out=outd[:, csl], in_=zl0[:])
        if stage >= 4:
            NMW = NCOL // D
            s_nm = [work.tile([D, NMW], f32, tag=f"snm{m}", name=f"snm{m}",
                              bufs=1) for m in range(P)]
            for m in range(P):
                nc.sync.dma_start(out=s_nm[m][:], in_=rows_t[32 * m:32 * m + 1, :])
            mx = work.tile([D, NMW], f32, tag="mx", name="mx")
            nc.vector.tensor_tensor(out=mx[:], in0=s_nm[0][:], in1=s_nm[1][:],
                                    op=mybir.AluOpType.max)
            nc.vector.tensor_tensor(out=mx[:], in0=mx[:], in1=s_nm[2][:],
                                    op=mybir.AluOpType.max)
            ex = [work.tile([D, NMW], f32, tag=f"ex{m}", name=f"ex{m}", bufs=1)
                  for m in range(P)]
            for m in range(P):
                d = work.tile([D, NMW], f32, tag="sd", name="sd")
                nc.vector.tensor_tensor(out=d[:], in0=s_nm[m][:], in1=mx[:],
                                        op=mybir.AluOpType.subtract)
                nc.scalar.activation(ex[m][:], d[:],
                                     mybir.ActivationFunctionType.Exp)
            sm = work.tile([D, NMW], f32, tag="sm", name="sm")
            nc.vector.tensor_tensor(out=sm[:], in0=ex[0][:], in1=ex[1][:],
                                    op=mybir.AluOpType.add)
            nc.vector.tensor_tensor(out=sm[:], in0=sm[:], in1=ex[2][:],
                                    op=mybir.AluOpType.add)
            rc = work.tile([D, NMW], f32, tag="rc", name="rc")
            nc.vector.reciprocal(out=rc[:], in_=sm[:])
            for m in range(P):
                bt = work.tile([D, NMW], f32, tag="bt", name="bt")
                nc.vector.tensor_tensor(out=bt[:], in0=ex[m][:], in1=rc[:],
                                        op=mybir.AluOpType.mult)
                nc.sync.dma_start(out=rows_t[32 * m:32 * m + 1, :], in_=bt[:])

            # ---- final combine per bank: out = sum_m beta_m * z_m + hT ----
            for b in range(NBANK):
                csl = slice(b * BANK, (b + 1) * BANK)
                acc = work.tile([D, BANK], f32, tag="acc", name="acc")
                tmp = work.tile([D, BANK], f32, tag="tmp", name="tmp")
                for m in range(P):
                    zl = work.tile([D, BANK], f32, tag="zl", name="zl", bufs=4)
                    nc.sync.dma_start(out=zl[:], in_=zspill[m, :, csl])
                    bps = ps_misc.tile([D, BANK], f32, space="PSUM", tag="fps", name="fps")
                    nc.tensor.matmul(out=bps[:], lhsT=ones_t[32 * m:32 * m + 1, :],
                                     rhs=rows_t[32 * m:32 * m + 1, csl],
                                     start=True, stop=True)
                    dst = acc if m == 0 else tmp
                    nc.vector.tensor_tensor(out=dst[:], in0=zl[:],
                                            in1=bps[:], op=mybir.AluOpType.mult)
                    if m > 0:
                        nc.vector.tensor_tensor(out=acc[:], in0=acc[:],
                                                in1=tmp[:],
                                                op=mybir.AluOpType.add)
                nc.vector.tensor_tensor(out=acc[:], in0=acc[:], in1=hT_t[:, csl],
                                        op=mybir.AluOpType.add)
                nc.sync.dma_start(out=outd[:, csl], in_=acc[:])

    nc.compile()
    return nc


# ---------------------------------------------------------------------------
# benchmarking (test-only; not used by the grading path)
# ---------------------------------------------------------------------------

def bench_exec_ns(nc, in_maps, iters_pairs=(2, 10), reps=3):
    """Estimate per-execution HW time by launching k async executions of
    the compiled program (PJRT pipelines them on-device) and differencing
    two k values (cancels the dispatch/round-trip constant)."""
    import time

    import jax
    from jax.experimental.shard_map import shard_map
    from jax.sharding import Mesh, PartitionSpec

    from concourse import mybir
    from concourse.bass2jax import (
        _bass_exec_p,
        install_neuronx_cc_hook,
        partition_id_tensor,
    )

    install_neuronx_cc_hook()
    n_cores = len(in_maps)
    partition_name = (
        nc.partition_id_tensor.name if nc.partition_id_tensor else None
    )
    in_names, out_names, out_avals, zero_outs = [], [], [], []
    for alloc in nc.m.functions[0].allocations:
        if not isinstance(alloc, mybir.MemoryLocationSet):
            continue
        name = alloc.memorylocations[0].name
        if alloc.kind == "ExternalInput":
            if name != partition_name:
                in_names.append(name)
        elif alloc.kind == "ExternalOutput":
            out_names.append(name)
            shape = tuple(alloc.tensor_shape)
            dtype = mybir.dt.np(alloc.dtype)
            out_avals.append(jax.core.ShapedArray(shape, dtype))
            zero_outs.append(np.zeros(shape, dtype))
    n_params = len(in_names)
    bind_names = tuple(
        in_names + out_names + ([partition_name] if partition_name else [])
    )

    def _body(*args):
        operands = list(args)
        if partition_name is not None:
            operands.append(partition_id_tensor())
        outs = _bass_exec_p.bind(
            *operands,
            out_avals=tuple(out_avals),
            in_names=bind_names,
            out_names=tuple(out_names),
            lowering_input_output_aliases=(),
            sim_require_finite=True,
            sim_require_nnan=True,
            nc=nc,
        )
        return tuple(outs)

    devices = jax.devices()[:n_cores]
    mesh = Mesh(np.asarray(devices), ("core",))
    fn = jax.jit(
        shard_map(
            _body,
            mesh=mesh,
            in_specs=(PartitionSpec("core"),) * (n_params + len(out_names)),
            out_specs=(PartitionSpec("core"),) * len(out_names),
            check_rep=False,
        )
    )

    concat_in = [
        np.concatenate([np.asarray(m[name]) for m in in_maps], axis=0)
        for name in in_names
    ]
    concat_zero = [
        np.zeros((n_cores * z.shape[0], *z.shape[1:]), z.dtype) for z in zero_outs
    ]
    dev_in = [jax.device_put(a) for a in concat_in]
    dev_zero = [jax.device_put(a) for a in concat_zero]

    jax.block_until_ready(fn(*dev_in, *dev_zero))  # compile + warm
    times = {}
    for k in sorted(set(iters_pairs)):
        best = float("inf")
        for _ in range(reps):
            t0 = time.perf_counter()
            outs = None
            for _i in range(k):
                outs = fn(*dev_in, *dev_zero)
            jax.block_until_ready(outs)
            best = min(best, time.perf_counter() - t0)
        times[k] = best
    k1, k2 = min(iters_pairs), max(iters_pairs)
    exec_ns = (times[k2] - times[k1]) / (k2 - k1) * 1e9
    return exec_ns, times


# ---------------------------------------------------------------------------
# entry point
# ---------------------------------------------------------------------------

def kernel(h, edge_rows, edge_cols, edge_vals, node_type,
           W_fc, prelu_a, Wg, bg, Wb, bb, film_bias,
           att_W1, att_b1, att_w2, _run_opts=None):
    _ensure_path()
    from concourse import bass_utils

    h = np.asarray(h, dtype=F32)
    edge_rows = np.asarray(edge_rows)
    edge_cols = np.asarray(edge_cols)
    edge_vals = np.asarray(edge_vals, dtype=F32)
    node_type = np.asarray(node_type)

    cfg, per_core = _plan(h, edge_rows, edge_cols, edge_vals, node_type)
    wmats, cvec = _pack_weights(cfg, np.asarray(W_fc), np.asarray(prelu_a),
                                np.asarray(Wg), np.asarray(bg),
                                np.asarray(Wb), np.asarray(bb),
                                np.asarray(film_bias), np.asarray(att_W1),
                                np.asarray(att_b1), np.asarray(att_w2))

    nc = _build_program(cfg, np.asarray(prelu_a, dtype=F32))

    npc = cfg["npc"]
    B0 = cfg["B0"]
    NCOL = cfg["NCOL"]
    in_maps = []
    for c in range(N_CORES):
        pc = per_core[c]
        hT_own = np.zeros((D, NCOL), dtype=F32)
        own = h[c * npc:(c + 1) * npc]       # [npc, D]
        srt = own[pc["perm"]]                 # type-sorted rows
        n0 = pc["n0"]
        hT_own[:, :n0] = srt[:n0].T
        hT_own[:, B0:B0 + (npc - n0)] = srt[n0:].T
        im = {
            "h_tab": h,
            "hT": hT_own,
            "wmats": wmats,
            "cvec": cvec,
        }
        for tag in ("L", "H"):
            for nm in ("idx", "doff", "val"):
                arr = pc[nm + tag]
                if arr.shape[1] == 0:  # empty stream: dram tensor padded to 1
                    arr = np.zeros(
                        (CHUNK, 1),
                        dtype=np.int16 if nm == "idx" else F32)
                    if nm == "idx":
                        arr -= 1
                im[nm + tag] = arr
        in_maps.append(im)

    run_kwargs = dict(_run_opts or {})
    bench = run_kwargs.pop("bench", None)
    run_kwargs.pop("_result", None)
    res = bass_utils.run_bass_kernel_spmd(
        nc, in_maps, core_ids=list(range(N_CORES)), **run_kwargs
    )
    if bench:
        exec_ns, times = bench_exec_ns(nc, in_maps)
        if isinstance(_run_opts, dict):
            _run_opts["_bench_ns"] = exec_ns
            _run_opts["_bench_times"] = times

    out = np.empty((cfg["N"], D), dtype=F32)
    for c in range(N_CORES):
        pc = per_core[c]
        n0 = pc["n0"]
        zT = res.results[c]["outT"]           # [D, NCOL]
        real = np.concatenate(
            [zT[:, :n0], zT[:, B0:B0 + (npc - n0)]], axis=1
        ).T                                    # [npc, D] sorted order
        shard = np.empty((npc, D), dtype=F32)
        shard[pc["perm"]] = real
        out[c * npc:(c + 1) * npc] = shard
    if isinstance(_run_opts, dict):
        _run_opts["_result"] = res
    return out



# revision 21
# speedup vs baseline: 15.3561x; 15.3561x over previous
"""MGNN (gnn_message_passing) Trainium2 kernel.

Strategy (8 NeuronCores, destination-sharded, no collectives):
  - Each core owns N/8 = 6250 destination nodes. Host partitions the edge
    lists by destination row, sorts by (local) destination, and pads edge
    chunks so all 8 cores run an identical SPMD program.
  - Aggregation identity: agg_i = segsum(val * (h @ W_i^T)[col])
                                = segsum(val * h[col]) @ W_i^T
    so the gather table is h itself for all 3 metapaths; the per-metapath
    weight matmul is applied after aggregation (on [D, n] tiles).
  - On device (feature-major layout [D=128 partitions, nodes on free dim]):
      * batched indirect-DMA gathers of h rows (bf16, 128 rows/chunk, KG
        chunks per DMA instruction); out-of-range pad indices are skipped.
      * segment-sum via one-hot matmul (bf16): S[e, d] = val_e *
        (iota[d]==doff_e), PSUM accumulation per 64-destination window.
      * FiLM folded into weights: gamma handled by sorting each core's
        nodes by type (host) and using two pre-scaled weight matrices.
        seq_fts residual is matmul-accumulated into the same PSUM tile.
      * PReLU(u) = max(u + bfb, a*u + a*bfb) via two scalar-engine affine
        ops + vector max; z kept resident in SBUF (bf16).
      * Semantics attention: tanh/score matmuls in feature-major form,
        softmax computed node-major after an SBUF reshape DMA.
  - Output is written feature-major bf16 [128, NCOL]; host transposes,
    strips padding, undoes the type-sort permutation, adds the +h residual
    in fp32 and concatenates shards.
"""

import math
import os

import numpy as np
import ml_dtypes

BF16 = ml_dtypes.bfloat16


def _ensure_path():
    try:
        import concourse  # noqa: F401
    except ImportError:
        import sys

        for p in ("/opt/trn_rl_repo", "/root/.axon_site/_ro/trn_rl_repo"):
            if os.path.isdir(p) and p not in sys.path:
                sys.path.insert(0, p)


# ---------------------------------------------------------------------------
# configuration
# ---------------------------------------------------------------------------

N_CORES = 8
D = 128          # hidden dim (= partition count)
CHUNK = 128      # edges per matmul chunk (contraction dim)
WIN = 64         # destinations per one-hot window (S width)
KG = 32          # chunks per dma_gather instruction
BANK = 512      # psum bank width (f32 elems) = 8 windows
PAD_COL = 1 << 28  # out-of-bounds gather index for pad edges (skipped)

F32 = np.float32
I32 = np.int32

# meta blob layout (int16 cols per gather batch): idx wrap + enc(f32) bits
MCOL = KG * 8 + KG * 2  # per-batch int16 columns: [idx | enc]


# ---------------------------------------------------------------------------
# custom DVE op: fused one-hot S build
#   S[p, k, w] = relu(d) * (d < 1),  d = enc[p, k] - w
#   where enc = doff + val packs the window offset (integer) and the edge
#   value (fraction) into one fp32; w is recovered on-engine from the
#   element position (Idx - PageIdx(0, WIN)).
# ---------------------------------------------------------------------------

_SEG_ONEHOT = None


def _get_seg_onehot():
    global _SEG_ONEHOT
    if _SEG_ONEHOT is not None:
        return _SEG_ONEHOT
    _ensure_path()
    from concourse import dve_ops
    from concourse.dve_ops import (
        _SUB_OPCODE_FOR_NAME,
        OPS,
        DveOp,
        has_src1,
        lower,
    )
    from concourse.dve_spec import C2, Idx, One, PageIdx, Spec, Src0, Zero, relu
    from concourse.dve_uop import DveOpSpec

    name = "SEG_ONEHOT_MGNN"
    if name in _SUB_OPCODE_FOR_NAME:
        _SEG_ONEHOT = next(o for o in OPS if o.name == name)
        return _SEG_ONEHOT

    d = Src0 - (Idx - PageIdx(Zero, C2))

    def _ref(in0, in1, s0, s1, imm2):
        in0 = np.asarray(in0, dtype=np.float32)
        S, N = in0.shape[-2], in0.shape[-1]
        idx = np.arange(S * N, dtype=np.float32).reshape(S, N)
        pg = (np.arange(S, dtype=np.float32) * imm2)[:, None]
        dd = in0 - (idx - pg)
        return (np.maximum(dd, 0.0) * (dd < 1.0)).astype(np.float32)

    spec = Spec(body=relu(d) * (d < One), reference=_ref)
    row = max(_SUB_OPCODE_FOR_NAME.values()) + 1
    assert row < 0x20
    shas = {}
    for ver in ("v3", "v4"):
        try:
            tmp = DveOpSpec(name=name, opcode=row, uops=lower(spec, ver=ver),
                            rd1_en=has_src1(spec))
            shas[ver] = tmp.sha(ver)
        except Exception:
            pass
    assert shas, "SEG_ONEHOT spec failed to lower"
    op = DveOp(name, spec, subdim=True, uops_sha=shas)
    _SUB_OPCODE_FOR_NAME[name] = row
    OPS.append(op)
    dve_ops.CUSTOM_DVE_SPECS[name] = spec
    _SEG_ONEHOT = op
    return op


# ---------------------------------------------------------------------------
# host-side planning
# ---------------------------------------------------------------------------

def _round_up(x, m):
    return (x + m - 1) // m * m


def _plan(h, edge_rows, edge_cols, edge_vals, node_type):
    """Shard by destination, type-sort each shard, build padded chunk plan.

    Chunks are segregated by source half (dma_gather indices are int16, so
    the gather table is split at NLO = N//2). Returns (cfg, per_core).
    """
    N = h.shape[0]
    P = edge_rows.shape[0]
    npc = N // N_CORES
    assert npc * N_CORES == N
    NLO = N // 2
    assert NLO <= 32768 and (N - NLO) <= 32768

    # --- per-core destination shards, sorted by node_type (stable) ---
    shards = []
    for c in range(N_CORES):
        own = slice(c * npc, (c + 1) * npc)
        t = node_type[own]
        perm = np.argsort(t, kind="stable")  # sorted-rank -> original local id
        n0 = int((t == 0).sum())
        shards.append({"perm": perm, "n0": n0})

    max_n0 = max(s["n0"] for s in shards)
    max_n1 = max(npc - s["n0"] for s in shards)
    B0 = _round_up(max(max_n0, 1), BANK)
    NCOL = B0 + _round_up(max(max_n1, 1), BANK)
    NBANK = NCOL // BANK
    NWIN = NCOL // WIN

    # padded-column map per core: local node id -> column
    for s in shards:
        inv = np.empty(npc, dtype=np.int64)
        inv[s["perm"]] = np.arange(npc)  # original local id -> sorted rank
        col = np.where(inv < s["n0"], inv, B0 + (inv - s["n0"]))
        s["colmap"] = col

    # --- edge bucketing by (core, metapath, half, window) ---
    edge_data = [[None] * P for _ in range(N_CORES)]
    hist = np.zeros((2, N_CORES, P, NWIN), dtype=np.int64)
    for c in range(N_CORES):
        base = c * npc
        for m in range(P):
            er = edge_rows[m]
            mask = (er >= base) & (er < base + npc)
            dl = shards[c]["colmap"][er[mask] - base]
            cs = edge_cols[m][mask].astype(np.int64)
            vs = edge_vals[m][mask].astype(F32)
            half = (cs >= NLO).astype(np.int64)
            # sort by (window, half) so each (w, half) group is contiguous
            key = (dl // WIN) * 2 + half
            order = np.argsort(key, kind="stable")
            dl = dl[order]
            cs = cs[order]
            vs = vs[order]
            half = half[order]
            w = dl // WIN
            for hf in range(2):
                hist[hf, c, m] += np.bincount(w[half == hf], minlength=NWIN)
            edge_data[c][m] = (dl, cs, vs, w, half)

    cl = np.maximum(1, -(-hist[0].max(axis=0) // CHUNK))   # [P, NWIN] lo
    ch = -(-hist[1].max(axis=0) // CHUNK)                  # [P, NWIN] hi
    counts2 = np.stack([cl, ch])                           # [2, P, NWIN]
    nch = [int(cl.sum()), int(ch.sum())]
    nch_pad = [_round_up(x, KG) for x in nch]

    # chunk slot base per (half, m, w) within its stream
    base_slot = np.zeros((2, P, NWIN), dtype=np.int64)
    for hf in range(2):
        flat = counts2[hf].reshape(-1)
        base_slot[hf].reshape(-1)[1:] = np.cumsum(flat)[:-1]

    per_core = []
    for c in range(N_CORES):
        streams = []
        for hf in range(2):
            nitems = nch_pad[hf] * CHUNK
            streams.append({
                "idx": np.full(nitems, -1, dtype=np.int64),
                "enc": np.zeros(nitems, dtype=F32),
            })
            # in-plan chunks: pad items default to row 0 / enc 0
            ninplan = int(counts2[hf].sum()) * CHUNK
            streams[hf]["idx"][:ninplan] = 0
        for m in range(P):
            dl, cs, vs, w, half = edge_data[c][m]
            for hf in range(2):
                sel = half == hf
                wm_ = w[sel]
                starts = np.searchsorted(wm_, np.arange(NWIN))
                rank = np.arange(len(wm_)) - starts[wm_]
                slot = base_slot[hf, m, wm_] * CHUNK + rank
                st = streams[hf]
                st["idx"][slot] = cs[sel] - hf * NLO
                # enc packs the window offset (int) + edge value (frac)
                st["enc"][slot] = (dl[sel] - wm_ * WIN).astype(F32) + vs[sel]
        pc = {"perm": shards[c]["perm"], "n0": shards[c]["n0"]}
        for hf, tag in ((0, "L"), (1, "H")):
            st = streams[hf]
            nb = nch_pad[hf] // KG
            # idx items wrapped in 16 partitions, replicated to 128
            iw = np.ascontiguousarray(
                st["idx"].reshape(-1, 16).T).astype(np.int16)   # [16, items/16]
            iw = np.tile(iw, (8, 1))                            # [128, items/16]
            enc = np.ascontiguousarray(
                st["enc"].reshape(-1, CHUNK).T)                 # [128, nch_pad]
            enc16 = enc.view(np.int16)                          # [128, 2*nch_pad]
            blob = np.empty((CHUNK, nb * MCOL), dtype=np.int16)
            for g in range(nb):
                blob[:, g * MCOL:g * MCOL + KG * 8] = \
                    iw[:, g * KG * 8:(g + 1) * KG * 8]
                blob[:, g * MCOL + KG * 8:(g + 1) * MCOL] = \
                    enc16[:, g * 2 * KG:(g + 1) * 2 * KG]
            pc["blob" + tag] = blob
        per_core.append(pc)

    # per-gather-batch valid index counts (same for all cores by construction)
    nreg = []
    for hf in range(2):
        ninplan = int(counts2[hf].sum())
        nb = nch_pad[hf] // KG
        r = []
        for g in range(nb):
            lo_c = g * KG
            r.append(max(0, min(ninplan - lo_c, KG)) * CHUNK)
        nreg.append(r)

    cfg = {
        "N": N,
        "NLO": NLO,
        "P": P,
        "npc": npc,
        "B0": B0,
        "NCOL": NCOL,
        "NBANK": NBANK,
        "NWIN": NWIN,
        "counts2": counts2,
        "nch_pad": nch_pad,
        "nreg": nreg,
    }
    return cfg, per_core


def _pack_weights(cfg, W_fc, prelu_a, Wg, bg, Wb, bb, film_bias,
                  att_W1, att_b1, att_w2):
    """Pack small weights into dense blobs (replicated to every core)."""
    P = cfg["P"]
    # wmats: per meta [W0T, W1T, WfcT], then att_W1T  -> [128, (3P+1)*128]
    blocks = []
    for m in range(P):
        g0 = (Wg[m][:, 0] + bg[m]).astype(F32)  # [D]
        g1 = (Wg[m][:, 1] + bg[m]).astype(F32)
        WT = W_fc[m].T.astype(F32)              # [fi, fo]
        blocks += [WT * g0[None, :], WT * g1[None, :], WT]
    blocks.append(att_W1.T.astype(F32))          # lhsT[d, hid]
    wmats = np.ascontiguousarray(
        np.concatenate(blocks, axis=1)).astype(BF16)

    # cvec (f32) [128, 16]: b1, per-meta (bfb0, bfb1, a*bfb0, a*bfb1)
    cvec = np.zeros((D, 16), dtype=F32)
    cvec[:, 0] = att_b1.astype(F32)
    for m in range(P):
        a = float(prelu_a[m])
        bfb0 = (Wb[m][:, 0] + bb[m] + film_bias[m]).astype(F32)
        bfb1 = (Wb[m][:, 1] + bb[m] + film_bias[m]).astype(F32)
        cvec[:, 2 + 4 * m] = bfb0
        cvec[:, 3 + 4 * m] = bfb1
        cvec[:, 4 + 4 * m] = a * bfb0
        cvec[:, 5 + 4 * m] = a * bfb1

    # cvec16 (bf16) [128, 2]: att_w2 (col 0)
    cvec16 = np.zeros((D, 2), dtype=BF16)
    cvec16[:, 0] = att_w2.astype(BF16)
    return wmats, cvec, cvec16


# ---------------------------------------------------------------------------
# device program
# ---------------------------------------------------------------------------

def _build_program(cfg, alphas, reps=1, ablate=()):
    _ensure_path()
    import concourse.bass as bass  # noqa: F401
    import concourse.tile as tile
    from concourse import bacc, mybir

    P = cfg["P"]
    NCOL = cfg["NCOL"]
    NBANK = cfg["NBANK"]
    counts2 = cfg["counts2"]
    nch_pad = cfg["nch_pad"]
    nreg = cfg["nreg"]
    N = cfg["N"]
    NLO = cfg["NLO"]
    dt = mybir.dt
    f32 = dt.float32
    bf16 = dt.bfloat16

    nc = bacc.Bacc(
        "TRN2",
        target_bir_lowering=False,
        debug=False,
        enable_asserts=False,
        num_devices=N_CORES,
    )

    h_tab = nc.dram_tensor("h_tab", [N, D], bf16, kind="ExternalInput").ap()
    hT = nc.dram_tensor("hT", [D, NCOL], bf16, kind="ExternalInput").ap()
    blobd = {}
    for hf, tag in ((0, "L"), (1, "H")):
        nb = max(nch_pad[hf] // KG, 1)
        blobd[hf] = nc.dram_tensor(f"blob{tag}", [CHUNK, nb * MCOL], dt.int16,
                                   kind="ExternalInput").ap()
    wmatsd = nc.dram_tensor("wmats", [D, (3 * P + 1) * D], bf16,
                            kind="ExternalInput").ap()
    cvecd = nc.dram_tensor("cvec", [D, 16], f32, kind="ExternalInput").ap()
    cvec16d = nc.dram_tensor("cvec16", [D, 2], bf16,
                             kind="ExternalInput").ap()
    outd = nc.dram_tensor("outT", [D, NCOL], bf16, kind="ExternalOutput").ap()

    half_tab = {0: h_tab[0:NLO, :], 1: h_tab[NLO:N, :]}

    with tile.TileContext(nc) as tc, tc.tile_pool(name="const", bufs=1) as cpool, \
            tc.tile_pool(name="gpool", bufs=2) as gpool, \
            tc.tile_pool(name="spool", bufs=2) as spool, \
            tc.tile_pool(name="mpool", bufs=2) as mpool, \
            tc.tile_pool(name="work", bufs=2) as work, \
            tc.tile_pool(name="ps_agg", bufs=3, space="PSUM") as ps_agg, \
            tc.tile_pool(name="ps_misc", bufs=2, space="PSUM") as ps_misc, \
            tc.tile_pool(name="ps_attn", bufs=2, space="PSUM") as ps_attn:
      for _rep in range(reps):
        # ---- constants / resident inputs ----
        hT_t = cpool.tile([D, NCOL], bf16, tag="hT", name="hT")
        nc.sync.dma_start(out=hT_t[:], in_=hT)
        wm_t = cpool.tile([D, (3 * P + 1) * D], bf16, tag="wm", name="wm")
        nc.sync.dma_start(out=wm_t[:], in_=wmatsd)
        cv_t = cpool.tile([D, 16], f32, tag="cv", name="cv")
        nc.sync.dma_start(out=cv_t[:], in_=cvecd)
        cv16_t = cpool.tile([D, 2], bf16, tag="cv16", name="cv16")
        nc.sync.dma_start(out=cv16_t[:], in_=cvec16d)
        ones_t = cpool.tile([65, D], bf16, tag="ones", name="ones")
        nc.vector.memset(ones_t[:], 1.0)

        def wmat(i):  # [128,128] lhsT block i
            return wm_t[:, i * D:(i + 1) * D]

        attW1T = wmat(3 * P)
        w2c = cv16_t[:, 0:1]
        b1c = cv_t[:, 0:1]

        # z embeddings, SBUF-resident (bf16), one tile per metapath
        z_t = [cpool.tile([D, NCOL], bf16, tag=f"z{m}", name=f"z{m}")
               for m in range(P)]
        # partitions 0/32/64 hold score rows s_m (f32)
        rows_t = cpool.tile([65, NCOL], f32, tag="rows", name="rows")
        # partitions 0/32/64 hold softmaxed beta_m (bf16)
        rows16_t = cpool.tile([65, NCOL], bf16, tag="rows16", name="rows16")

        # ---- gather + S build, two half streams ----
        # (no pre-zero needed: every chunk consumed by a matmul is fully
        # gather-written — in-plan pad edges use idx=0; only trailing
        # rounding chunks are unwritten and they are never read)
        gtiles = {}
        stiles = {}
        onehot_op = _get_seg_onehot()

        def ensure_batch(hf, g):
            if (hf, g) in gtiles:
                return
            tg = "gL" if hf == 0 else "gH"
            mb = mpool.tile([CHUNK, MCOL], dt.int16, tag="mb" + tg,
                            name="mb" + tg)
            nc.sync.dma_start(out=mb[:],
                              in_=blobd[hf][:, g * MCOL:(g + 1) * MCOL])
            ix = mb[:, 0:KG * 8]
            menc = mb[:, KG * 8:MCOL].bitcast(f32)   # [CHUNK, KG]
            gt = gpool.tile([CHUNK, KG * D], bf16, tag=tg, name=tg)
            if "nogather" not in ablate:
                nc.gpsimd.dma_gather(
                    out_ap=gt[:].rearrange("p (k d) -> p k d", k=KG),
                    in_ap=half_tab[hf],
                    idxs_ap=ix,
                    num_idxs=KG * CHUNK,
                    num_idxs_reg=int(nreg[hf][g]),
                    elem_size=D,
                    single_packet=False,
                )
            st = spool.tile([CHUNK, KG * WIN], bf16, tag="s" + tg,
                            name="s" + tg)
            nc.vector._custom_dve(
                onehot_op,
                out=st[:].rearrange("p (k w) -> p k w", k=KG),
                in0=menc.unsqueeze(2).to_broadcast([CHUNK, KG, WIN]),
                imm2=float(WIN),
            )
            gtiles[(hf, g)] = gt
            stiles[(hf, g)] = st

        chunk_c = [0, 0]
        WPB = BANK // WIN  # windows per bank
        pending_tail = [None]

        def flush_tail():
            if pending_tail[0] is not None:
                pending_tail[0]()
                pending_tail[0] = None

        def make_tail(m, b, aggh):
            def tail():
                # z_pre^T = W_t . aggh + W . hT   (accumulated in PSUM)
                fps = ps_misc.tile([D, BANK], f32, space="PSUM", tag="fps", name="fps")
                wsel = 3 * m + (0 if b < cfg["B0"] // BANK else 1)
                csl = slice(b * BANK, (b + 1) * BANK)
                nc.tensor.matmul(out=fps[:], lhsT=wmat(wsel), rhs=aggh[:],
                                 start=True, stop=False)
                nc.tensor.matmul(out=fps[:], lhsT=wmat(3 * m + 2),
                                 rhs=hT_t[:, csl], start=False, stop=True)
                # PReLU(u + bfb) = max(u + bfb, a*u + a*bfb)
                ty = 0 if b < cfg["B0"] // BANK else 1
                bfb = cv_t[:, 2 + 4 * m + ty:3 + 4 * m + ty]
                abfb = cv_t[:, 4 + 4 * m + ty:5 + 4 * m + ty]
                t0 = work.tile([D, BANK], bf16, tag="t0", name="t0")
                t1 = work.tile([D, BANK], bf16, tag="t1", name="t1")
                nc.scalar.activation(t0[:], fps[:],
                                     mybir.ActivationFunctionType.Identity,
                                     bias=bfb, scale=1.0)
                nc.scalar.activation(t1[:], fps[:],
                                     mybir.ActivationFunctionType.Identity,
                                     bias=abfb, scale=float(alphas[m]))
                nc.vector.tensor_tensor(out=z_t[m][:, csl], in0=t0[:],
                                        in1=t1[:], op=mybir.AluOpType.max)
                # attention scores for this bank
                aps = ps_attn.tile([D, BANK], f32, space="PSUM", tag="at", name="at")
                nc.tensor.matmul(out=aps[:], lhsT=attW1T, rhs=z_t[m][:, csl],
                                 start=True, stop=True)
                th = work.tile([D, BANK], bf16, tag="tanh", name="tanh")
                nc.scalar.activation(th[:], aps[:],
                                     mybir.ActivationFunctionType.Tanh,
                                     bias=b1c, scale=1.0)
                sps = ps_attn.tile([1, BANK], f32, space="PSUM", tag="at", name="at")
                nc.tensor.matmul(out=sps[:], lhsT=w2c, rhs=th[:],
                                 start=True, stop=True)
                nc.scalar.copy(out=rows_t[32 * m:32 * m + 1, csl], in_=sps[:])
            return tail

        for m in range(P):
            for b in range(NBANK):
                agg = ps_agg.tile([D, BANK], f32, space="PSUM", tag="agg", name="agg")
                for wl in range(WPB):
                    w = b * WPB + wl
                    tot = int(counts2[0, m, w]) + int(counts2[1, m, w])
                    j = 0
                    for hf in range(2):
                        for _ in range(int(counts2[hf, m, w])):
                            g, cl = divmod(chunk_c[hf], KG)
                            ensure_batch(hf, g)
                            nc.tensor.matmul(
                                out=agg[:, wl * WIN:(wl + 1) * WIN],
                                lhsT=gtiles[(hf, g)][:, cl * D:(cl + 1) * D],
                                rhs=stiles[(hf, g)][:, cl * WIN:(cl + 1) * WIN],
                                start=(j == 0),
                                stop=(j == tot - 1),
                            )
                            chunk_c[hf] += 1
                            j += 1
                # previous bank's tail: its inputs are ready by now, so the
                # in-order engines never stall on fresh PSUM/Act results
                flush_tail()
                # evacuate A@h bank to SBUF as bf16 (scalar engine copy)
                aggh = work.tile([D, BANK], bf16, tag="aggh", name="aggh")
                nc.scalar.copy(out=aggh[:], in_=agg[:])
                if "notail" in ablate:
                    if m == P - 1:
                        nc.sync.dma_start(out=outd[:, slice(b * BANK, (b + 1) * BANK)],
                                          in_=aggh[:])
                    continue
                pending_tail[0] = make_tail(m, b, aggh)
        flush_tail()

        assert chunk_c[0] == int(counts2[0].sum())
        assert chunk_c[1] == int(counts2[1].sum())
        if "notail" in ablate:
            continue

        # ---- softmax over metapaths (node-major [128, NCOL/128]) ----
        NMW = NCOL // D
        s_nm = [work.tile([D, NMW], f32, tag=f"snm{m}", name=f"snm{m}",
                          bufs=1) for m in range(P)]
        for m in range(P):
            nc.sync.dma_start(out=s_nm[m][:], in_=rows_t[32 * m:32 * m + 1, :])
        mx = work.tile([D, NMW], f32, tag="mx", name="mx")
        nc.vector.tensor_tensor(out=mx[:], in0=s_nm[0][:], in1=s_nm[1][:],
                                op=mybir.AluOpType.max)
        nc.vector.tensor_tensor(out=mx[:], in0=mx[:], in1=s_nm[2][:],
                                op=mybir.AluOpType.max)
        ex = [work.tile([D, NMW], f32, tag=f"ex{m}", name=f"ex{m}", bufs=1)
              for m in range(P)]
        for m in range(P):
            d = work.tile([D, NMW], f32, tag="sd", name="sd")
            nc.vector.tensor_tensor(out=d[:], in0=s_nm[m][:], in1=mx[:],
                                    op=mybir.AluOpType.subtract)
            nc.scalar.activation(ex[m][:], d[:],
                                 mybir.ActivationFunctionType.Exp)
        sm = work.tile([D, NMW], f32, tag="sm", name="sm")
        nc.vector.tensor_tensor(out=sm[:], in0=ex[0][:], in1=ex[1][:],
                                op=mybir.AluOpType.add)
        nc.vector.tensor_tensor(out=sm[:], in0=sm[:], in1=ex[2][:],
                                op=mybir.AluOpType.add)
        rc = work.tile([D, NMW], f32, tag="rc", name="rc")
        nc.vector.reciprocal(out=rc[:], in_=sm[:])
        for m in range(P):
            bt = work.tile([D, NMW], bf16, tag="bt", name="bt")
            nc.vector.tensor_tensor(out=bt[:], in0=ex[m][:], in1=rc[:],
                                    op=mybir.AluOpType.mult)
            nc.sync.dma_start(out=rows16_t[32 * m:32 * m + 1, :], in_=bt[:])

        # ---- final combine per bank: out = sum_m beta_m * z_m  (+h on host)
        for b in range(NBANK):
            csl = slice(b * BANK, (b + 1) * BANK)
            acc = work.tile([D, BANK], bf16, tag="acc", name="acc")
            tmp = work.tile([D, BANK], bf16, tag="tmp", name="tmp")
            for m in range(P):
                bps = ps_misc.tile([D, BANK], f32, space="PSUM", tag="fps", name="fps")
                nc.tensor.matmul(out=bps[:], lhsT=ones_t[32 * m:32 * m + 1, :],
                                 rhs=rows16_t[32 * m:32 * m + 1, csl],
                                 start=True, stop=True)
                bc16 = work.tile([D, BANK], bf16, tag="bc16", name="bc16")
                nc.scalar.copy(out=bc16[:], in_=bps[:])
                dst = acc if m == 0 else tmp
                nc.vector.tensor_tensor(out=dst[:], in0=z_t[m][:, csl],
                                        in1=bc16[:], op=mybir.AluOpType.mult)
                if m > 0:
                    nc.vector.tensor_tensor(out=acc[:], in0=acc[:],
                                            in1=tmp[:],
                                            op=mybir.AluOpType.add)
            nc.sync.dma_start(out=outd[:, csl], in_=acc[:])

    nc.compile()
    return nc


# ---------------------------------------------------------------------------
# benchmarking (test-only; not used by the grading path)
# ---------------------------------------------------------------------------

def _make_runner(nc, in_maps):
    """Build a jitted runner for a prebuilt program with device-resident
    inputs; returns (fn, dev_args) where fn(*dev_args) executes once."""
    import jax
    from jax.experimental.shard_map import shard_map
    from jax.sharding import Mesh, PartitionSpec

    from concourse import mybir
    from concourse.bass2jax import (
        _bass_exec_p,
        install_neuronx_cc_hook,
        partition_id_tensor,
    )

    install_neuronx_cc_hook()
    n_cores = len(in_maps)
    partition_name = (
        nc.partition_id_tensor.name if nc.partition_id_tensor else None
    )
    in_names, out_names, out_avals, zero_outs = [], [], [], []
    for alloc in nc.m.functions[0].allocations:
        if not isinstance(alloc, mybir.MemoryLocationSet):
            continue
        name = alloc.memorylocations[0].name
        if alloc.kind == "ExternalInput":
            if name != partition_name:
                in_names.append(name)
        elif alloc.kind == "ExternalOutput":
            out_names.append(name)
            shape = tuple(alloc.tensor_shape)
            dtype = mybir.dt.np(alloc.dtype)
            out_avals.append(jax.core.ShapedArray(shape, dtype))
            zero_outs.append(np.zeros(shape, dtype))
    n_params = len(in_names)
    bind_names = tuple(
        in_names + out_names + ([partition_name] if partition_name else [])
    )

    def _body(*args):
        operands = list(args)
        if partition_name is not None:
            operands.append(partition_id_tensor())
        outs = _bass_exec_p.bind(
            *operands,
            out_avals=tuple(out_avals),
            in_names=bind_names,
            out_names=tuple(out_names),
            lowering_input_output_aliases=(),
            sim_require_finite=True,
            sim_require_nnan=True,
            nc=nc,
        )
        return tuple(outs)

    devices = jax.devices()[:n_cores]
    mesh = Mesh(np.asarray(devices), ("core",))
    fn = jax.jit(
        shard_map(
            _body,
            mesh=mesh,
            in_specs=(PartitionSpec("core"),) * (n_params + len(out_names)),
            out_specs=(PartitionSpec("core"),) * len(out_names),
            check_rep=False,
        )
    )
    concat_in = [
        np.concatenate([np.asarray(m[name]) for m in in_maps], axis=0)
        for name in in_names
    ]
    concat_zero = [
        np.zeros((n_cores * z.shape[0], *z.shape[1:]), z.dtype) for z in zero_outs
    ]
    dev_args = [jax.device_put(a) for a in concat_in + concat_zero]
    return fn, dev_args


def bench_exec_ns(cfg, alphas, in_maps, nc1, reps_hi=41, timing_reps=7):
    """Difference wall time of a 1-rep vs reps_hi-rep program (kernel body
    repeated inside one NEFF) to cancel the dispatch round trip."""
    import time

    import jax

    nc_hi = _build_program(cfg, alphas, reps=reps_hi)
    results = {}
    for label, nc in (("lo", nc1), ("hi", nc_hi)):
        fn, dev_args = _make_runner(nc, in_maps)
        jax.block_until_ready(fn(*dev_args))  # compile + warm
        best = float("inf")
        for _ in range(timing_reps):
            t0 = time.perf_counter()
            jax.block_until_ready(fn(*dev_args))
            best = min(best, time.perf_counter() - t0)
        results[label] = best
    exec_ns = (results["hi"] - results["lo"]) / (reps_hi - 1) * 1e9
    return exec_ns, results


# ---------------------------------------------------------------------------
# entry point
# ---------------------------------------------------------------------------

def kernel(h, edge_rows, edge_cols, edge_vals, node_type,
           W_fc, prelu_a, Wg, bg, Wb, bb, film_bias,
           att_W1, att_b1, att_w2, _run_opts=None):
    _ensure_path()
    from concourse import bass_utils

    h = np.asarray(h, dtype=F32)
    edge_rows = np.asarray(edge_rows)
    edge_cols = np.asarray(edge_cols)
    edge_vals = np.asarray(edge_vals, dtype=F32)
    node_type = np.asarray(node_type)

    cfg, per_core = _plan(h, edge_rows, edge_cols, edge_vals, node_type)
    wmats, cvec, cvec16 = _pack_weights(
        cfg, np.asarray(W_fc), np.asarray(prelu_a),
        np.asarray(Wg), np.asarray(bg),
        np.asarray(Wb), np.asarray(bb),
        np.asarray(film_bias), np.asarray(att_W1),
        np.asarray(att_b1), np.asarray(att_w2))

    nc = _build_program(cfg, np.asarray(prelu_a, dtype=F32))

    npc = cfg["npc"]
    B0 = cfg["B0"]
    NCOL = cfg["NCOL"]
    h16 = h.astype(BF16)
    in_maps = []
    for c in range(N_CORES):
        pc = per_core[c]
        hT_own = np.zeros((D, NCOL), dtype=BF16)
        own = h16[c * npc:(c + 1) * npc]      # [npc, D]
        srt = own[pc["perm"]]                 # type-sorted rows
        n0 = pc["n0"]
        hT_own[:, :n0] = srt[:n0].T
        hT_own[:, B0:B0 + (npc - n0)] = srt[n0:].T
        im = {
            "h_tab": h16,
            "hT": hT_own,
            "wmats": wmats,
            "cvec": cvec,
            "cvec16": cvec16,
        }
        for tag in ("L", "H"):
            arr = pc["blob" + tag]
            if arr.shape[1] == 0:  # empty stream: dram tensor padded
                arr = np.full((CHUNK, MCOL), -1, dtype=np.int16)
            im["blob" + tag] = arr
        in_maps.append(im)

    run_kwargs = dict(_run_opts or {})
    bench = run_kwargs.pop("bench", None)
    run_kwargs.pop("_result", None)
    run_kwargs.pop("_bench_ns", None)
    run_kwargs.pop("_bench_times", None)
    res = bass_utils.run_bass_kernel_spmd(
        nc, in_maps, core_ids=list(range(N_CORES)), **run_kwargs
    )

    out = np.empty((cfg["N"], D), dtype=F32)
    for c in range(N_CORES):
        pc = per_core[c]
        n0 = pc["n0"]
        zT = res.results[c]["outT"].astype(F32)   # [D, NCOL] bf16 -> f32
        real = np.concatenate(
            [zT[:, :n0], zT[:, B0:B0 + (npc - n0)]], axis=1
        ).T                                    # [npc, D] sorted order
        shard = np.empty((npc, D), dtype=F32)
        shard[pc["perm"]] = real
        out[c * npc:(c + 1) * npc] = shard + h[c * npc:(c + 1) * npc]

    if bench:
        exec_ns, times = bench_exec_ns(
            cfg, np.asarray(prelu_a, dtype=F32), in_maps, nc)
        if isinstance(_run_opts, dict):
            _run_opts["_bench_ns"] = exec_ns
            _run_opts["_bench_times"] = times
    if isinstance(_run_opts, dict):
        _run_opts["_result"] = res
    return out


# revision 46
# speedup vs baseline: 35.3086x; 2.2993x over previous
"""MGNN (gnn_message_passing) Trainium2 kernel.

Strategy (8 NeuronCores, destination-sharded, no collectives):
  - Each core owns N/8 = 6250 destination nodes. Host partitions the edge
    lists by destination row, sorts by (local) destination, and pads edge
    chunks so all 8 cores run an identical SPMD program.
  - Aggregation identity: agg_i = segsum(val * (h @ W_i^T)[col])
                                = segsum(val * h[col]) @ W_i^T
    so the gather table is h itself for all 3 metapaths; the per-metapath
    weight matmul is applied after aggregation (on [D, n] tiles).
  - On device (feature-major layout [D=128 partitions, nodes on free dim]):
      * batched indirect-DMA gathers of h rows (bf16, 128 rows/chunk, KG
        chunks per DMA instruction); out-of-range pad indices are skipped.
      * segment-sum via one-hot matmul (bf16): S[e, d] = val_e *
        (iota[d]==doff_e), PSUM accumulation per 64-destination window.
      * FiLM folded into weights: gamma handled by sorting each core's
        nodes by type (host) and using two pre-scaled weight matrices.
        seq_fts residual is matmul-accumulated into the same PSUM tile.
      * PReLU(u) = max(u + bfb, a*u + a*bfb) via two scalar-engine affine
        ops + vector max; z kept resident in SBUF (bf16).
      * Semantics attention: tanh/score matmuls in feature-major form,
        softmax computed node-major after an SBUF reshape DMA.
  - Output is written feature-major bf16 [128, NCOL]; host transposes,
    strips padding, undoes the type-sort permutation, adds the +h residual
    in fp32 and concatenates shards.
"""

import math
import os

import numpy as np
import ml_dtypes

BF16 = ml_dtypes.bfloat16


def _ensure_path():
    try:
        import concourse  # noqa: F401
    except ImportError:
        import sys

        for p in ("/opt/trn_rl_repo", "/root/.axon_site/_ro/trn_rl_repo"):
            if os.path.isdir(p) and p not in sys.path:
                sys.path.insert(0, p)


# ---------------------------------------------------------------------------
# configuration
# ---------------------------------------------------------------------------

N_CORES = 8
D = 128          # hidden dim (= partition count)
CHUNK = 128      # edges per matmul chunk (contraction dim)
WIN = 64         # destinations per one-hot window (S width)
KG = 32          # chunks per dma_gather instruction
BANK = 512      # psum bank width (f32 elems) = 8 windows
PAD_COL = 1 << 28  # out-of-bounds gather index for pad edges (skipped)

F32 = np.float32
I32 = np.int32

# meta blob layout (int16 cols per gather batch): idx wrap + enc(f32) bits
MCOL = KG * 8 + KG * 2  # per-batch int16 columns: [idx | enc]


# ---------------------------------------------------------------------------
# custom DVE op: fused one-hot S build
#   S[p, k, w] = relu(d) * (d < 1),  d = enc[p, k] - w
#   where enc = doff + val packs the window offset (integer) and the edge
#   value (fraction) into one fp32; w is recovered on-engine from the
#   element position (Idx - PageIdx(0, WIN)).
# ---------------------------------------------------------------------------

_SEG_ONEHOT = None


def _get_seg_onehot():
    global _SEG_ONEHOT
    if _SEG_ONEHOT is not None:
        return _SEG_ONEHOT
    _ensure_path()
    from concourse import dve_ops
    from concourse.dve_ops import (
        _SUB_OPCODE_FOR_NAME,
        OPS,
        DveOp,
        has_src1,
        lower,
    )
    from concourse.dve_spec import C2, Idx, One, PageIdx, Spec, Src0, Zero, relu
    from concourse.dve_uop import DveOpSpec

    name = "SEG_ONEHOT_MGNN"
    if name in _SUB_OPCODE_FOR_NAME:
        _SEG_ONEHOT = next(o for o in OPS if o.name == name)
        return _SEG_ONEHOT

    d = Src0 - (Idx - PageIdx(Zero, C2))

    def _ref(in0, in1, s0, s1, imm2):
        in0 = np.asarray(in0, dtype=np.float32)
        S, N = in0.shape[-2], in0.shape[-1]
        idx = np.arange(S * N, dtype=np.float32).reshape(S, N)
        pg = (np.arange(S, dtype=np.float32) * imm2)[:, None]
        dd = in0 - (idx - pg)
        return (np.maximum(dd, 0.0) * (dd < 1.0)).astype(np.float32)

    spec = Spec(body=relu(d) * (d < One), reference=_ref)
    row = max(_SUB_OPCODE_FOR_NAME.values()) + 1
    assert row < 0x20
    shas = {}
    for ver in ("v3", "v4"):
        try:
            tmp = DveOpSpec(name=name, opcode=row, uops=lower(spec, ver=ver),
                            rd1_en=has_src1(spec))
            shas[ver] = tmp.sha(ver)
        except Exception:
            pass
    assert shas, "SEG_ONEHOT spec failed to lower"
    op = DveOp(name, spec, subdim=True, uops_sha=shas)
    _SUB_OPCODE_FOR_NAME[name] = row
    OPS.append(op)
    dve_ops.CUSTOM_DVE_SPECS[name] = spec
    _SEG_ONEHOT = op
    return op


# ---------------------------------------------------------------------------
# host-side planning
# ---------------------------------------------------------------------------

def _round_up(x, m):
    return (x + m - 1) // m * m


def _round_up_arr(a, m):
    return (a + m - 1) // m * m


def _plan(h, edge_rows, edge_cols, edge_vals, node_type):
    """Shard by destination, type-sort each shard, build padded chunk plan.

    Chunks are segregated by source half (dma_gather indices are int16, so
    the gather table is split at NLO = N//2). Returns (cfg, per_core).
    """
    N = h.shape[0]
    P = edge_rows.shape[0]
    npc = N // N_CORES
    assert npc * N_CORES == N
    NLO = N // 2
    assert NLO <= 32768 and (N - NLO) <= 32768

    # --- per-core destination shards, sorted by node_type (stable) ---
    shards = []
    for c in range(N_CORES):
        own = slice(c * npc, (c + 1) * npc)
        t = node_type[own]
        perm = np.argsort(t, kind="stable")  # sorted-rank -> original local id
        n0 = int((t == 0).sum())
        shards.append({"perm": perm, "n0": n0})

    max_n0 = max(s["n0"] for s in shards)
    max_n1 = max(npc - s["n0"] for s in shards)
    B0 = _round_up(max(max_n0, 1), BANK)
    NCOL = B0 + _round_up(max(max_n1, 1), BANK)
    NBANK = NCOL // BANK
    NWIN = NCOL // WIN

    # padded-column map per core: local node id -> column
    for s in shards:
        inv = np.empty(npc, dtype=np.int64)
        inv[s["perm"]] = np.arange(npc)  # original local id -> sorted rank
        col = np.where(inv < s["n0"], inv, B0 + (inv - s["n0"]))
        s["colmap"] = col

    # --- edge bucketing by (core, metapath, half, window) ---
    edge_data = [[None] * P for _ in range(N_CORES)]
    hist = np.zeros((2, N_CORES, P, NWIN), dtype=np.int64)
    for c in range(N_CORES):
        base = c * npc
        for m in range(P):
            er = edge_rows[m]
            mask = (er >= base) & (er < base + npc)
            dl = shards[c]["colmap"][er[mask] - base]
            cs = edge_cols[m][mask].astype(np.int64)
            vs = edge_vals[m][mask].astype(F32)
            half = (cs >= NLO).astype(np.int64)
            # sort by (window, half) so each (w, half) group is contiguous
            key = (dl // WIN) * 2 + half
            order = np.argsort(key, kind="stable")
            dl = dl[order]
            cs = cs[order]
            vs = vs[order]
            half = half[order]
            w = dl // WIN
            for hf in range(2):
                hist[hf, c, m] += np.bincount(w[half == hf], minlength=NWIN)
            edge_data[c][m] = (dl, cs, vs, w, half)

    QNT = 128  # group padding quantum (multiple of 32; 128 = whole chunks)
    gl = np.maximum(QNT, _round_up_arr(hist[0].max(axis=0), QNT))  # [P, NWIN]
    gh = _round_up_arr(hist[1].max(axis=0), QNT)
    gsz = np.stack([gl, gh])               # [2, P, NWIN] padded slot counts

    # slot base per (half, m, w); matmul base partition must be 0/32/64, so
    # lead-pad any nonzero group that would start at slot 96 (mod 128)
    base_slot = np.zeros((2, P, NWIN), dtype=np.int64)
    tot_slots = [0, 0]
    for hf in range(2):
        flat = gsz[hf].reshape(-1)
        bflat = base_slot[hf].reshape(-1)
        base = 0
        for i in range(flat.size):
            if flat[i] > 0 and base % CHUNK == 96:
                base += 32
            bflat[i] = base
            base += int(flat[i])
        tot_slots[hf] = base
    nch_pad = [_round_up(-(-t // CHUNK), KG) for t in tot_slots]

    # matmul pieces per (m, w): (hf, chunk, p0, p1) — 32-aligned partition
    # sub-ranges of gather chunks, in stream order
    pieces = [[[] for _ in range(NWIN)] for _ in range(P)]
    for m in range(P):
        for w in range(NWIN):
            for hf in range(2):
                s0 = int(base_slot[hf, m, w])
                s1 = s0 + int(gsz[hf, m, w])
                s = s0
                while s < s1:
                    p0 = s % CHUNK
                    # PE operand partition ranges: base 0 -> <=128,
                    # base 32 -> <=32, base 64 -> <=64 (base 96 illegal)
                    cap = 128 if p0 == 0 else (32 if p0 == 32 else 64)
                    e = min(s1, s + cap)
                    pieces[m][w].append((hf, s // CHUNK, p0, p0 + (e - s)))
                    s = e

    per_core = []
    for c in range(N_CORES):
        streams = []
        for hf in range(2):
            nitems = nch_pad[hf] * CHUNK
            streams.append({
                "idx": np.full(nitems, -1, dtype=np.int64),
                "enc": np.zeros(nitems, dtype=F32),
            })
            # in-plan slots: pad items default to row 0 / enc 0
            streams[hf]["idx"][:tot_slots[hf]] = 0
        for m in range(P):
            dl, cs, vs, w, half = edge_data[c][m]
            for hf in range(2):
                sel = half == hf
                wm_ = w[sel]
                starts = np.searchsorted(wm_, np.arange(NWIN))
                rank = np.arange(len(wm_)) - starts[wm_]
                slot = base_slot[hf, m, wm_] + rank
                st = streams[hf]
                st["idx"][slot] = cs[sel] - hf * NLO
                # enc packs the window offset (int) + edge value (frac)
                st["enc"][slot] = (dl[sel] - wm_ * WIN).astype(F32) + vs[sel]
        pc = {"perm": shards[c]["perm"], "n0": shards[c]["n0"]}
        for hf, tag in ((0, "L"), (1, "H")):
            st = streams[hf]
            nb = nch_pad[hf] // KG
            # idx items wrapped in 16 partitions, replicated to 128
            iw = np.ascontiguousarray(
                st["idx"].reshape(-1, 16).T).astype(np.int16)   # [16, items/16]
            iw = np.tile(iw, (8, 1))                            # [128, items/16]
            enc = np.ascontiguousarray(
                st["enc"].reshape(-1, CHUNK).T)                 # [128, nch_pad]
            enc16 = enc.view(np.int16)                          # [128, 2*nch_pad]
            blob = np.empty((CHUNK, nb * MCOL), dtype=np.int16)
            for g in range(nb):
                blob[:, g * MCOL:g * MCOL + KG * 8] = \
                    iw[:, g * KG * 8:(g + 1) * KG * 8]
                blob[:, g * MCOL + KG * 8:(g + 1) * MCOL] = \
                    enc16[:, g * 2 * KG:(g + 1) * 2 * KG]
            pc["blob" + tag] = blob
        per_core.append(pc)

    # per-gather-batch valid index counts (same for all cores by construction)
    nreg = []
    for hf in range(2):
        nb = nch_pad[hf] // KG
        r = []
        for g in range(nb):
            r.append(max(0, min(tot_slots[hf] - g * KG * CHUNK, KG * CHUNK)))
        nreg.append(r)

    cfg = {
        "N": N,
        "NLO": NLO,
        "P": P,
        "npc": npc,
        "B0": B0,
        "NCOL": NCOL,
        "NBANK": NBANK,
        "NWIN": NWIN,
        "pieces": pieces,
        "tot_slots": tot_slots,
        "nch_pad": nch_pad,
        "nreg": nreg,
    }
    return cfg, per_core


def _pack_weights(cfg, W_fc, prelu_a, Wg, bg, Wb, bb, film_bias,
                  att_W1, att_b1, att_w2):
    """Pack small weights into dense blobs (replicated to every core)."""
    P = cfg["P"]
    # wmats: per meta [W0T, W1T, WfcT], then att_W1T  -> [128, (3P+1)*128]
    blocks = []
    for m in range(P):
        g0 = (Wg[m][:, 0] + bg[m]).astype(F32)  # [D]
        g1 = (Wg[m][:, 1] + bg[m]).astype(F32)
        WT = W_fc[m].T.astype(F32)              # [fi, fo]
        blocks += [WT * g0[None, :], WT * g1[None, :], WT]
    blocks.append(att_W1.T.astype(F32))          # lhsT[d, hid]
    wmats = np.ascontiguousarray(
        np.concatenate(blocks, axis=1)).astype(BF16)

    # cvec (f32) [128, 16]: b1, per-meta (bfb0, bfb1, a*bfb0, a*bfb1)
    cvec = np.zeros((D, 16), dtype=F32)
    cvec[:, 0] = att_b1.astype(F32)
    for m in range(P):
        a = float(prelu_a[m])
        bfb0 = (Wb[m][:, 0] + bb[m] + film_bias[m]).astype(F32)
        bfb1 = (Wb[m][:, 1] + bb[m] + film_bias[m]).astype(F32)
        cvec[:, 2 + 4 * m] = bfb0
        cvec[:, 3 + 4 * m] = bfb1
        cvec[:, 4 + 4 * m] = a * bfb0
        cvec[:, 5 + 4 * m] = a * bfb1

    # cvec16 (bf16) [128, 2]: att_w2 (col 0)
    cvec16 = np.zeros((D, 2), dtype=BF16)
    cvec16[:, 0] = att_w2.astype(BF16)
    return wmats, cvec, cvec16


# ---------------------------------------------------------------------------
# device program
# ---------------------------------------------------------------------------

def _build_program(cfg, alphas, reps=1, ablate=(), queues=4, single_packet=False,
                   scratch=16384, gbufs=3):
    _ensure_path()
    import concourse.bass as bass  # noqa: F401
    import concourse.tile as tile
    from concourse import bacc, mybir

    P = cfg["P"]
    NCOL = cfg["NCOL"]
    NBANK = cfg["NBANK"]
    pieces = cfg["pieces"]
    nch_pad = cfg["nch_pad"]
    nreg = cfg["nreg"]
    N = cfg["N"]
    NLO = cfg["NLO"]
    dt = mybir.dt
    f32 = dt.float32
    bf16 = dt.bfloat16

    nc = bacc.Bacc(
        "TRN2",
        target_bir_lowering=False,
        debug=False,
        enable_asserts=False,
        num_devices=N_CORES,
        num_swdge_queues=queues,
        dynamic_dma_scratch_size=scratch,
    )
    batch_count = [0]

    h_tab = nc.dram_tensor("h_tab", [N, D], bf16, kind="ExternalInput").ap()
    hT = nc.dram_tensor("hT", [D, NCOL], bf16, kind="ExternalInput").ap()
    blobd = {}
    for hf, tag in ((0, "L"), (1, "H")):
        nb = max(nch_pad[hf] // KG, 1)
        blobd[hf] = nc.dram_tensor(f"blob{tag}", [CHUNK, nb * MCOL], dt.int16,
                                   kind="ExternalInput").ap()
    wmatsd = nc.dram_tensor("wmats", [D, (3 * P + 1) * D], bf16,
                            kind="ExternalInput").ap()
    cvecd = nc.dram_tensor("cvec", [D, 16], f32, kind="ExternalInput").ap()
    cvec16d = nc.dram_tensor("cvec16", [D, 2], bf16,
                             kind="ExternalInput").ap()
    outd = nc.dram_tensor("outT", [D, NCOL], bf16, kind="ExternalOutput").ap()

    half_tab = {0: h_tab[0:NLO, :], 1: h_tab[NLO:N, :]}

    with tile.TileContext(nc) as tc, tc.tile_pool(name="const", bufs=1) as cpool, \
            tc.tile_pool(name="gpool", bufs=gbufs) as gpool, \
            tc.tile_pool(name="spool", bufs=gbufs) as spool, \
            tc.tile_pool(name="mpool", bufs=gbufs) as mpool, \
            tc.tile_pool(name="work", bufs=2) as work, \
            tc.tile_pool(name="ps_agg", bufs=3, space="PSUM") as ps_agg, \
            tc.tile_pool(name="ps_misc", bufs=2, space="PSUM") as ps_misc, \
            tc.tile_pool(name="ps_attn", bufs=2, space="PSUM") as ps_attn:
      for _rep in range(reps):
        # ---- constants / resident inputs ----
        hT_t = cpool.tile([D, NCOL], bf16, tag="hT", name="hT")
        nc.sync.dma_start(out=hT_t[:], in_=hT)
        wm_t = cpool.tile([D, (3 * P + 1) * D], bf16, tag="wm", name="wm")
        nc.sync.dma_start(out=wm_t[:], in_=wmatsd)
        cv_t = cpool.tile([D, 16], f32, tag="cv", name="cv")
        nc.sync.dma_start(out=cv_t[:], in_=cvecd)
        cv16_t = cpool.tile([D, 2], bf16, tag="cv16", name="cv16")
        nc.sync.dma_start(out=cv16_t[:], in_=cvec16d)
        ones_t = cpool.tile([65, D], bf16, tag="ones", name="ones")
        nc.vector.memset(ones_t[:], 1.0)

        def wmat(i):  # [128,128] lhsT block i
            return wm_t[:, i * D:(i + 1) * D]

        attW1T = wmat(3 * P)
        w2c = cv16_t[:, 0:1]
        b1c = cv_t[:, 0:1]

        # z embeddings, SBUF-resident (bf16), one tile per metapath
        z_t = [cpool.tile([D, NCOL], bf16, tag=f"z{m}", name=f"z{m}")
               for m in range(P)]
        # partitions 0/32/64 hold score rows s_m (f32)
        rows_t = cpool.tile([65, NCOL], f32, tag="rows", name="rows")
        # partitions 0/32/64 hold softmaxed beta_m (bf16)
        rows16_t = cpool.tile([65, NCOL], bf16, tag="rows16", name="rows16")

        # ---- gather + S build, two half streams ----
        # (no pre-zero needed: every chunk consumed by a matmul is fully
        # gather-written — in-plan pad edges use idx=0; only trailing
        # rounding chunks are unwritten and they are never read)
        gtiles = {}
        stiles = {}
        onehot_op = _get_seg_onehot()

        def ensure_batch(hf, g):
            if (hf, g) in gtiles:
                return
            tg = "gL" if hf == 0 else "gH"
            mb = mpool.tile([CHUNK, MCOL], dt.int16, tag="mb" + tg,
                            name="mb" + tg)
            nc.sync.dma_start(out=mb[:],
                              in_=blobd[hf][:, g * MCOL:(g + 1) * MCOL])
            ix = mb[:, 0:KG * 8]
            menc = mb[:, KG * 8:MCOL].bitcast(f32)   # [CHUNK, KG]
            gt = gpool.tile([CHUNK, KG * D], bf16, tag=tg, name=tg)
            if "nogather" not in ablate:
                nc.gpsimd.dma_gather(
                    out_ap=gt[:].rearrange("p (k d) -> p k d", k=KG),
                    in_ap=half_tab[hf],
                    idxs_ap=ix,
                    num_idxs=KG * CHUNK,
                    num_idxs_reg=int(nreg[hf][g]),
                    elem_size=D,
                    single_packet=single_packet,
                    queue_num=batch_count[0] % queues,
                )
            else:
                nc.vector.memset(gt[:, 0:1], 0.0)  # cheap writer stub
            batch_count[0] += 1
            st = spool.tile([CHUNK, KG * WIN], bf16, tag="s" + tg,
                            name="s" + tg)
            if "nos" not in ablate:
                nc.vector._custom_dve(
                    onehot_op,
                    out=st[:].rearrange("p (k w) -> p k w", k=KG),
                    in0=menc.unsqueeze(2).to_broadcast([CHUNK, KG, WIN]),
                    imm2=float(WIN),
                )
            gtiles[(hf, g)] = gt
            stiles[(hf, g)] = st

        WPB = BANK // WIN  # windows per bank
        pending_tail = [None]

        def flush_tail():
            if pending_tail[0] is not None:
                pending_tail[0]()
                pending_tail[0] = None

        def make_tail(m, b, aggh):
            def tail():
                # z_pre^T = W_t . aggh + W . hT   (accumulated in PSUM)
                fps = ps_misc.tile([D, BANK], f32, space="PSUM", tag="fps", name="fps")
                wsel = 3 * m + (0 if b < cfg["B0"] // BANK else 1)
                csl = slice(b * BANK, (b + 1) * BANK)
                nc.tensor.matmul(out=fps[:], lhsT=wmat(wsel), rhs=aggh[:],
                                 start=True, stop=False)
                nc.tensor.matmul(out=fps[:], lhsT=wmat(3 * m + 2),
                                 rhs=hT_t[:, csl], start=False, stop=True)
                # PReLU(u + bfb) = max(u + bfb, a*u + a*bfb)
                ty = 0 if b < cfg["B0"] // BANK else 1
                bfb = cv_t[:, 2 + 4 * m + ty:3 + 4 * m + ty]
                abfb = cv_t[:, 4 + 4 * m + ty:5 + 4 * m + ty]
                t0 = work.tile([D, BANK], bf16, tag="t0", name="t0")
                t1 = work.tile([D, BANK], bf16, tag="t1", name="t1")
                nc.scalar.activation(t0[:], fps[:],
                                     mybir.ActivationFunctionType.Identity,
                                     bias=bfb, scale=1.0)
                nc.scalar.activation(t1[:], fps[:],
                                     mybir.ActivationFunctionType.Identity,
                                     bias=abfb, scale=float(alphas[m]))
                nc.vector.tensor_tensor(out=z_t[m][:, csl], in0=t0[:],
                                        in1=t1[:], op=mybir.AluOpType.max)
                # attention scores for this bank
                aps = ps_attn.tile([D, BANK], f32, space="PSUM", tag="at", name="at")
                nc.tensor.matmul(out=aps[:], lhsT=attW1T, rhs=z_t[m][:, csl],
                                 start=True, stop=True)
                th = work.tile([D, BANK], bf16, tag="tanh", name="tanh")
                nc.scalar.activation(th[:], aps[:],
                                     mybir.ActivationFunctionType.Tanh,
                                     bias=b1c, scale=1.0)
                sps = ps_attn.tile([1, BANK], f32, space="PSUM", tag="at", name="at")
                nc.tensor.matmul(out=sps[:], lhsT=w2c, rhs=th[:],
                                 start=True, stop=True)
                nc.scalar.copy(out=rows_t[32 * m:32 * m + 1, csl], in_=sps[:])
            return tail

        NMB = BANK // D

        def softmax_combine(b):
            # per-bank softmax over metapaths (node-major [128, BANK/128])
            # + combine: out = sum_m beta_m * z_m  (+h on host). Emitted as
            # soon as all three metapath scores for bank b exist, so it
            # pipelines under the remaining m=P-1 gather stream.
            csl = slice(b * BANK, (b + 1) * BANK)
            s_nm = [work.tile([D, NMB], f32, tag=f"snm{m}", name=f"snm{m}")
                    for m in range(P)]
            for m in range(P):
                nc.sync.dma_start(out=s_nm[m][:],
                                  in_=rows_t[32 * m:32 * m + 1, csl])
            mx = work.tile([D, NMB], f32, tag="mx", name="mx")
            nc.vector.tensor_tensor(out=mx[:], in0=s_nm[0][:], in1=s_nm[1][:],
                                    op=mybir.AluOpType.max)
            nc.vector.tensor_tensor(out=mx[:], in0=mx[:], in1=s_nm[2][:],
                                    op=mybir.AluOpType.max)
            ex = [work.tile([D, NMB], f32, tag=f"ex{m}", name=f"ex{m}")
                  for m in range(P)]
            for m in range(P):
                sd = work.tile([D, NMB], f32, tag="sd", name="sd")
                nc.vector.tensor_tensor(out=sd[:], in0=s_nm[m][:], in1=mx[:],
                                        op=mybir.AluOpType.subtract)
                nc.scalar.activation(ex[m][:], sd[:],
                                     mybir.ActivationFunctionType.Exp)
            sm = work.tile([D, NMB], f32, tag="sm", name="sm")
            nc.vector.tensor_tensor(out=sm[:], in0=ex[0][:], in1=ex[1][:],
                                    op=mybir.AluOpType.add)
            nc.vector.tensor_tensor(out=sm[:], in0=sm[:], in1=ex[2][:],
                                    op=mybir.AluOpType.add)
            rc = work.tile([D, NMB], f32, tag="rc", name="rc")
            nc.vector.reciprocal(out=rc[:], in_=sm[:])
            acc = work.tile([D, BANK], bf16, tag="acc", name="acc")
            tmp = work.tile([D, BANK], bf16, tag="tmp", name="tmp")
            for m in range(P):
                bt = work.tile([D, NMB], bf16, tag="bt", name="bt")
                nc.vector.tensor_tensor(out=bt[:], in0=ex[m][:], in1=rc[:],
                                        op=mybir.AluOpType.mult)
                nc.sync.dma_start(out=rows16_t[32 * m:32 * m + 1, csl],
                                  in_=bt[:])
                bps = ps_misc.tile([D, BANK], f32, space="PSUM", tag="fps",
                                   name="fps")
                nc.tensor.matmul(out=bps[:], lhsT=ones_t[32 * m:32 * m + 1, :],
                                 rhs=rows16_t[32 * m:32 * m + 1, csl],
                                 start=True, stop=True)
                bc16 = work.tile([D, BANK], bf16, tag="bc16", name="bc16")
                nc.scalar.copy(out=bc16[:], in_=bps[:])
                dst = acc if m == 0 else tmp
                nc.vector.tensor_tensor(out=dst[:], in0=z_t[m][:, csl],
                                        in1=bc16[:], op=mybir.AluOpType.mult)
                if m > 0:
                    nc.vector.tensor_tensor(out=acc[:], in0=acc[:],
                                            in1=tmp[:],
                                            op=mybir.AluOpType.add)
            nc.sync.dma_start(out=outd[:, csl], in_=acc[:])

        for m in range(P):
            for b in range(NBANK):
                agg = ps_agg.tile([D, BANK], f32, space="PSUM", tag="agg", name="agg")
                for wl in range(WPB):
                    w = b * WPB + wl
                    pl = pieces[m][w]
                    for j, (hf, ch, p0, p1) in enumerate(pl):
                        g, cl = divmod(ch, KG)
                        ensure_batch(hf, g)
                        if "nomm" not in ablate:
                            nc.tensor.matmul(
                                out=agg[:, wl * WIN:(wl + 1) * WIN],
                                lhsT=gtiles[(hf, g)][p0:p1, cl * D:(cl + 1) * D],
                                rhs=stiles[(hf, g)][p0:p1,
                                                    cl * WIN:(cl + 1) * WIN],
                                start=(j == 0),
                                stop=(j == len(pl) - 1),
                            )
                # previous bank's tail: its inputs are ready by now, so the
                # in-order engines never stall on fresh PSUM/Act results
                flush_tail()
                if "nomm" in ablate:
                    if m == P - 1:
                        csl2 = slice(b * BANK, (b + 1) * BANK)
                        nc.sync.dma_start(out=outd[:, csl2], in_=hT_t[:, csl2])
                    continue
                # evacuate A@h bank to SBUF as bf16 (scalar engine copy)
                aggh = work.tile([D, BANK], bf16, tag="aggh", name="aggh")
                nc.scalar.copy(out=aggh[:], in_=agg[:])
                if "notail" in ablate:
                    if m == P - 1:
                        nc.sync.dma_start(out=outd[:, slice(b * BANK, (b + 1) * BANK)],
                                          in_=aggh[:])
                    continue
                pending_tail[0] = make_tail(m, b, aggh)
                if m == P - 1 and b >= 1:
                    softmax_combine(b - 1)
        flush_tail()

        if "notail" in ablate or "nomm" in ablate:
            continue

        softmax_combine(NBANK - 1)

    nc.compile()
    return nc


# ---------------------------------------------------------------------------
# benchmarking (test-only; not used by the grading path)
# ---------------------------------------------------------------------------

def _make_runner(nc, in_maps):
    """Build a jitted runner for a prebuilt program with device-resident
    inputs; returns (fn, dev_args) where fn(*dev_args) executes once."""
    import jax
    from jax.experimental.shard_map import shard_map
    from jax.sharding import Mesh, PartitionSpec

    from concourse import mybir
    from concourse.bass2jax import (
        _bass_exec_p,
        install_neuronx_cc_hook,
        partition_id_tensor,
    )

    install_neuronx_cc_hook()
    n_cores = len(in_maps)
    partition_name = (
        nc.partition_id_tensor.name if nc.partition_id_tensor else None
    )
    in_names, out_names, out_avals, zero_outs = [], [], [], []
    for alloc in nc.m.functions[0].allocations:
        if not isinstance(alloc, mybir.MemoryLocationSet):
            continue
        name = alloc.memorylocations[0].name
        if alloc.kind == "ExternalInput":
            if name != partition_name:
                in_names.append(name)
        elif alloc.kind == "ExternalOutput":
            out_names.append(name)
            shape = tuple(alloc.tensor_shape)
            dtype = mybir.dt.np(alloc.dtype)
            out_avals.append(jax.core.ShapedArray(shape, dtype))
            zero_outs.append(np.zeros(shape, dtype))
    n_params = len(in_names)
    bind_names = tuple(
        in_names + out_names + ([partition_name] if partition_name else [])
    )

    def _body(*args):
        operands = list(args)
        if partition_name is not None:
            operands.append(partition_id_tensor())
        outs = _bass_exec_p.bind(
            *operands,
            out_avals=tuple(out_avals),
            in_names=bind_names,
            out_names=tuple(out_names),
            lowering_input_output_aliases=(),
            sim_require_finite=True,
            sim_require_nnan=True,
            nc=nc,
        )
        return tuple(outs)

    devices = jax.devices()[:n_cores]
    mesh = Mesh(np.asarray(devices), ("core",))
    fn = jax.jit(
        shard_map(
            _body,
            mesh=mesh,
            in_specs=(PartitionSpec("core"),) * (n_params + len(out_names)),
            out_specs=(PartitionSpec("core"),) * len(out_names),
            check_rep=False,
        )
    )
    concat_in = [
        np.concatenate([np.asarray(m[name]) for m in in_maps], axis=0)
        for name in in_names
    ]
    concat_zero = [
        np.zeros((n_cores * z.shape[0], *z.shape[1:]), z.dtype) for z in zero_outs
    ]
    dev_args = [jax.device_put(a) for a in concat_in + concat_zero]
    return fn, dev_args


def bench_exec_ns(cfg, alphas, in_maps, nc1, reps_hi=41, timing_reps=7):
    """Difference wall time of a 1-rep vs reps_hi-rep program (kernel body
    repeated inside one NEFF) to cancel the dispatch round trip."""
    import time

    import jax

    nc_hi = _build_program(cfg, alphas, reps=reps_hi)
    results = {}
    for label, nc in (("lo", nc1), ("hi", nc_hi)):
        fn, dev_args = _make_runner(nc, in_maps)
        jax.block_until_ready(fn(*dev_args))  # compile + warm
        best = float("inf")
        for _ in range(timing_reps):
            t0 = time.perf_counter()
            jax.block_until_ready(fn(*dev_args))
            best = min(best, time.perf_counter() - t0)
        results[label] = best
    exec_ns = (results["hi"] - results["lo"]) / (reps_hi - 1) * 1e9
    return exec_ns, results


# ---------------------------------------------------------------------------
# entry point
# ---------------------------------------------------------------------------

def kernel(h, edge_rows, edge_cols, edge_vals, node_type,
           W_fc, prelu_a, Wg, bg, Wb, bb, film_bias,
           att_W1, att_b1, att_w2, _run_opts=None):
    _ensure_path()
    from concourse import bass_utils

    h = np.asarray(h, dtype=F32)
    edge_rows = np.asarray(edge_rows)
    edge_cols = np.asarray(edge_cols)
    edge_vals = np.asarray(edge_vals, dtype=F32)
    node_type = np.asarray(node_type)

    cfg, per_core = _plan(h, edge_rows, edge_cols, edge_vals, node_type)
    wmats, cvec, cvec16 = _pack_weights(
        cfg, np.asarray(W_fc), np.asarray(prelu_a),
        np.asarray(Wg), np.asarray(bg),
        np.asarray(Wb), np.asarray(bb),
        np.asarray(film_bias), np.asarray(att_W1),
        np.asarray(att_b1), np.asarray(att_w2))

    nc = _build_program(cfg, np.asarray(prelu_a, dtype=F32))

    npc = cfg["npc"]
    B0 = cfg["B0"]
    NCOL = cfg["NCOL"]
    h16 = h.astype(BF16)
    in_maps = []
    for c in range(N_CORES):
        pc = per_core[c]
        hT_own = np.zeros((D, NCOL), dtype=BF16)
        own = h16[c * npc:(c + 1) * npc]      # [npc, D]
        srt = own[pc["perm"]]                 # type-sorted rows
        n0 = pc["n0"]
        hT_own[:, :n0] = srt[:n0].T
        hT_own[:, B0:B0 + (npc - n0)] = srt[n0:].T
        im = {
            "h_tab": h16,
            "hT": hT_own,
            "wmats": wmats,
            "cvec": cvec,
            "cvec16": cvec16,
        }
        for tag in ("L", "H"):
            arr = pc["blob" + tag]
            if arr.shape[1] == 0:  # empty stream: dram tensor padded
                arr = np.full((CHUNK, MCOL), -1, dtype=np.int16)
            im["blob" + tag] = arr
        in_maps.append(im)

    run_kwargs = dict(_run_opts or {})
    bench = run_kwargs.pop("bench", None)
    run_kwargs.pop("_result", None)
    run_kwargs.pop("_bench_ns", None)
    run_kwargs.pop("_bench_times", None)
    res = bass_utils.run_bass_kernel_spmd(
        nc, in_maps, core_ids=list(range(N_CORES)), **run_kwargs
    )

    out = np.empty((cfg["N"], D), dtype=F32)
    for c in range(N_CORES):
        pc = per_core[c]
        n0 = pc["n0"]
        zT = res.results[c]["outT"].astype(F32)   # [D, NCOL] bf16 -> f32
        real = np.concatenate(
            [zT[:, :n0], zT[:, B0:B0 + (npc - n0)]], axis=1
        ).T                                    # [npc, D] sorted order
        shard = np.empty((npc, D), dtype=F32)
        shard[pc["perm"]] = real
        out[c * npc:(c + 1) * npc] = shard + h[c * npc:(c + 1) * npc]

    if bench:
        exec_ns, times = bench_exec_ns(
            cfg, np.asarray(prelu_a, dtype=F32), in_maps, nc)
        if isinstance(_run_opts, dict):
            _run_opts["_bench_ns"] = exec_ns
            _run_opts["_bench_times"] = times
    if isinstance(_run_opts, dict):
        _run_opts["_result"] = res
    return out


# revision 50
# speedup vs baseline: 96.4037x; 2.7303x over previous
"""MGNN (gnn_message_passing) Trainium2 kernel.

Strategy (8 NeuronCores, destination-sharded, no collectives):
  - Each core owns N/8 = 6250 destination nodes. Host partitions the edge
    lists by destination row, sorts by (local) destination, and pads edge
    chunks so all 8 cores run an identical SPMD program.
  - Aggregation identity: agg_i = segsum(val * (h @ W_i^T)[col])
                                = segsum(val * h[col]) @ W_i^T
    so the gather table is h itself for all 3 metapaths; the per-metapath
    weight matmul is applied after aggregation (on [D, n] tiles).
  - On device (feature-major layout [D=128 partitions, nodes on free dim]):
      * batched indirect-DMA gathers of h rows (bf16, 128 rows/chunk, KG
        chunks per DMA instruction); out-of-range pad indices are skipped.
      * segment-sum via one-hot matmul (bf16): S[e, d] = val_e *
        (iota[d]==doff_e), PSUM accumulation per 64-destination window.
      * FiLM folded into weights: gamma handled by sorting each core's
        nodes by type (host) and using two pre-scaled weight matrices.
        seq_fts residual is matmul-accumulated into the same PSUM tile.
      * PReLU(u) = max(u + bfb, a*u + a*bfb) via two scalar-engine affine
        ops + vector max; z kept resident in SBUF (bf16).
      * Semantics attention: tanh/score matmuls in feature-major form,
        softmax computed node-major after an SBUF reshape DMA.
  - Output is written feature-major bf16 [128, NCOL]; host transposes,
    strips padding, undoes the type-sort permutation, adds the +h residual
    in fp32 and concatenates shards.
"""

import math
import os

import numpy as np
import ml_dtypes

BF16 = ml_dtypes.bfloat16


def _ensure_path():
    try:
        import concourse  # noqa: F401
    except ImportError:
        import sys

        for p in ("/opt/trn_rl_repo", "/root/.axon_site/_ro/trn_rl_repo"):
            if os.path.isdir(p) and p not in sys.path:
                sys.path.insert(0, p)


# ---------------------------------------------------------------------------
# configuration
# ---------------------------------------------------------------------------

N_CORES = 8
D = 128          # hidden dim (= partition count)
CHUNK = 128      # edges per matmul chunk (contraction dim)
WIN = 64         # destinations per one-hot window (S width)
KG = 32          # chunks per dma_gather instruction
BANK = 512      # psum bank width (f32 elems) = 8 windows
PAD_COL = 1 << 28  # out-of-bounds gather index for pad edges (skipped)

F32 = np.float32
I32 = np.int32

# meta blob layout (int16 cols per gather batch): idx wrap + enc(f32) bits
MCOL = KG * 8 + KG * 2  # per-batch int16 columns: [idx | enc]


# ---------------------------------------------------------------------------
# custom DVE op: fused one-hot S build
#   S[p, k, w] = relu(d) * (d < 1),  d = enc[p, k] - w
#   where enc = doff + val packs the window offset (integer) and the edge
#   value (fraction) into one fp32; w is recovered on-engine from the
#   element position (Idx - PageIdx(0, WIN)).
# ---------------------------------------------------------------------------

_SEG_ONEHOT = None


def _get_seg_onehot():
    global _SEG_ONEHOT
    if _SEG_ONEHOT is not None:
        return _SEG_ONEHOT
    _ensure_path()
    from concourse import dve_ops
    from concourse.dve_ops import (
        _SUB_OPCODE_FOR_NAME,
        OPS,
        DveOp,
        has_src1,
        lower,
    )
    from concourse.dve_spec import C2, Idx, One, PageIdx, Spec, Src0, Zero, relu
    from concourse.dve_uop import DveOpSpec

    name = "SEG_ONEHOT_MGNN"
    if name in _SUB_OPCODE_FOR_NAME:
        _SEG_ONEHOT = next(o for o in OPS if o.name == name)
        return _SEG_ONEHOT

    d = Src0 - (Idx - PageIdx(Zero, C2))

    def _ref(in0, in1, s0, s1, imm2):
        in0 = np.asarray(in0, dtype=np.float32)
        S, N = in0.shape[-2], in0.shape[-1]
        idx = np.arange(S * N, dtype=np.float32).reshape(S, N)
        pg = (np.arange(S, dtype=np.float32) * imm2)[:, None]
        dd = in0 - (idx - pg)
        return (np.maximum(dd, 0.0) * (dd < 1.0)).astype(np.float32)

    spec = Spec(body=relu(d) * (d < One), reference=_ref)
    row = max(_SUB_OPCODE_FOR_NAME.values()) + 1
    assert row < 0x20
    shas = {}
    for ver in ("v3", "v4"):
        try:
            tmp = DveOpSpec(name=name, opcode=row, uops=lower(spec, ver=ver),
                            rd1_en=has_src1(spec))
            shas[ver] = tmp.sha(ver)
        except Exception:
            pass
    assert shas, "SEG_ONEHOT spec failed to lower"
    op = DveOp(name, spec, subdim=True, uops_sha=shas)
    _SUB_OPCODE_FOR_NAME[name] = row
    OPS.append(op)
    dve_ops.CUSTOM_DVE_SPECS[name] = spec
    _SEG_ONEHOT = op
    return op


# ---------------------------------------------------------------------------
# host-side planning
# ---------------------------------------------------------------------------

def _round_up(x, m):
    return (x + m - 1) // m * m


def _round_up_arr(a, m):
    return (a + m - 1) // m * m


def _plan(h, edge_rows, edge_cols, edge_vals, node_type):
    """Shard by destination, type-sort each shard, build padded chunk plan.

    Chunks are segregated by source half (dma_gather indices are int16, so
    the gather table is split at NLO = N//2). Returns (cfg, per_core).
    """
    N = h.shape[0]
    P = edge_rows.shape[0]
    npc = N // N_CORES
    assert npc * N_CORES == N
    NLO = N // 2
    assert NLO <= 32768 and (N - NLO) <= 32768

    # --- per-core destination shards, sorted by node_type (stable) ---
    shards = []
    for c in range(N_CORES):
        own = slice(c * npc, (c + 1) * npc)
        t = node_type[own]
        perm = np.argsort(t, kind="stable")  # sorted-rank -> original local id
        n0 = int((t == 0).sum())
        shards.append({"perm": perm, "n0": n0})

    max_n0 = max(s["n0"] for s in shards)
    max_n1 = max(npc - s["n0"] for s in shards)
    B0 = _round_up(max(max_n0, 1), BANK)
    NCOL = B0 + _round_up(max(max_n1, 1), BANK)
    NBANK = NCOL // BANK
    NWIN = NCOL // WIN

    # padded-column map per core: local node id -> column
    for s in shards:
        inv = np.empty(npc, dtype=np.int64)
        inv[s["perm"]] = np.arange(npc)  # original local id -> sorted rank
        col = np.where(inv < s["n0"], inv, B0 + (inv - s["n0"]))
        s["colmap"] = col

    # --- edge bucketing by (core, metapath, half, window) ---
    edge_data = [[None] * P for _ in range(N_CORES)]
    hist = np.zeros((2, N_CORES, P, NWIN), dtype=np.int64)
    for c in range(N_CORES):
        base = c * npc
        for m in range(P):
            er = edge_rows[m]
            mask = (er >= base) & (er < base + npc)
            dl = shards[c]["colmap"][er[mask] - base]
            cs = edge_cols[m][mask].astype(np.int64)
            vs = edge_vals[m][mask].astype(F32)
            half = (cs >= NLO).astype(np.int64)
            # sort by (window, half) so each (w, half) group is contiguous
            key = (dl // WIN) * 2 + half
            order = np.argsort(key, kind="stable")
            dl = dl[order]
            cs = cs[order]
            vs = vs[order]
            half = half[order]
            w = dl // WIN
            for hf in range(2):
                hist[hf, c, m] += np.bincount(w[half == hf], minlength=NWIN)
            edge_data[c][m] = (dl, cs, vs, w, half)

    QNT = 128  # group padding quantum (multiple of 32; 128 = whole chunks)
    gl = np.maximum(QNT, _round_up_arr(hist[0].max(axis=0), QNT))  # [P, NWIN]
    gh = _round_up_arr(hist[1].max(axis=0), QNT)
    gsz = np.stack([gl, gh])               # [2, P, NWIN] padded slot counts

    # slot base per (half, m, w); matmul base partition must be 0/32/64, so
    # lead-pad any nonzero group that would start at slot 96 (mod 128)
    base_slot = np.zeros((2, P, NWIN), dtype=np.int64)
    tot_slots = [0, 0]
    for hf in range(2):
        flat = gsz[hf].reshape(-1)
        bflat = base_slot[hf].reshape(-1)
        base = 0
        for i in range(flat.size):
            if flat[i] > 0 and base % CHUNK == 96:
                base += 32
            bflat[i] = base
            base += int(flat[i])
        tot_slots[hf] = base
    nch_pad = [_round_up(-(-t // CHUNK), KG) for t in tot_slots]

    # matmul pieces per (m, w): (hf, chunk, p0, p1) — 32-aligned partition
    # sub-ranges of gather chunks, in stream order
    pieces = [[[] for _ in range(NWIN)] for _ in range(P)]
    for m in range(P):
        for w in range(NWIN):
            for hf in range(2):
                s0 = int(base_slot[hf, m, w])
                s1 = s0 + int(gsz[hf, m, w])
                s = s0
                while s < s1:
                    p0 = s % CHUNK
                    # PE operand partition ranges: base 0 -> <=128,
                    # base 32 -> <=32, base 64 -> <=64 (base 96 illegal)
                    cap = 128 if p0 == 0 else (32 if p0 == 32 else 64)
                    e = min(s1, s + cap)
                    pieces[m][w].append((hf, s // CHUNK, p0, p0 + (e - s)))
                    s = e

    per_core = []
    for c in range(N_CORES):
        streams = []
        for hf in range(2):
            nitems = nch_pad[hf] * CHUNK
            streams.append({
                "idx": np.full(nitems, -1, dtype=np.int64),
                "enc": np.zeros(nitems, dtype=F32),
            })
            # in-plan slots: pad items default to row 0 / enc 0
            streams[hf]["idx"][:tot_slots[hf]] = 0
        for m in range(P):
            dl, cs, vs, w, half = edge_data[c][m]
            for hf in range(2):
                sel = half == hf
                wm_ = w[sel]
                starts = np.searchsorted(wm_, np.arange(NWIN))
                rank = np.arange(len(wm_)) - starts[wm_]
                slot = base_slot[hf, m, wm_] + rank
                st = streams[hf]
                st["idx"][slot] = cs[sel] - hf * NLO
                # enc packs the window offset (int) + edge value (frac)
                st["enc"][slot] = (dl[sel] - wm_ * WIN).astype(F32) + vs[sel]
        pc = {"perm": shards[c]["perm"], "n0": shards[c]["n0"]}
        for hf, tag in ((0, "L"), (1, "H")):
            st = streams[hf]
            nb = nch_pad[hf] // KG
            # idx items wrapped in 16 partitions, replicated to 128
            iw = np.ascontiguousarray(
                st["idx"].reshape(-1, 16).T).astype(np.int16)   # [16, items/16]
            iw = np.tile(iw, (8, 1))                            # [128, items/16]
            enc = np.ascontiguousarray(
                st["enc"].reshape(-1, CHUNK).T)                 # [128, nch_pad]
            enc16 = enc.view(np.int16)                          # [128, 2*nch_pad]
            blob = np.empty((CHUNK, nb * MCOL), dtype=np.int16)
            for g in range(nb):
                blob[:, g * MCOL:g * MCOL + KG * 8] = \
                    iw[:, g * KG * 8:(g + 1) * KG * 8]
                blob[:, g * MCOL + KG * 8:(g + 1) * MCOL] = \
                    enc16[:, g * 2 * KG:(g + 1) * 2 * KG]
            pc["blob" + tag] = blob
        per_core.append(pc)

    # per-gather-batch valid index counts (same for all cores by construction)
    nreg = []
    for hf in range(2):
        nb = nch_pad[hf] // KG
        r = []
        for g in range(nb):
            r.append(max(0, min(tot_slots[hf] - g * KG * CHUNK, KG * CHUNK)))
        nreg.append(r)

    cfg = {
        "N": N,
        "NLO": NLO,
        "P": P,
        "npc": npc,
        "B0": B0,
        "NCOL": NCOL,
        "NBANK": NBANK,
        "NWIN": NWIN,
        "pieces": pieces,
        "tot_slots": tot_slots,
        "nch_pad": nch_pad,
        "nreg": nreg,
    }
    return cfg, per_core


def _pack_weights(cfg, W_fc, prelu_a, Wg, bg, Wb, bb, film_bias,
                  att_W1, att_b1, att_w2):
    """Pack small weights into dense blobs (replicated to every core)."""
    P = cfg["P"]
    # wmats: per meta [W0T, W1T, WfcT], then att_W1T  -> [128, (3P+1)*128]
    blocks = []
    for m in range(P):
        g0 = (Wg[m][:, 0] + bg[m]).astype(F32)  # [D]
        g1 = (Wg[m][:, 1] + bg[m]).astype(F32)
        WT = W_fc[m].T.astype(F32)              # [fi, fo]
        blocks += [WT * g0[None, :], WT * g1[None, :], WT]
    blocks.append(att_W1.T.astype(F32))          # lhsT[d, hid]
    wmats = np.ascontiguousarray(
        np.concatenate(blocks, axis=1)).astype(BF16)

    # cvec (f32) [128, 16]: b1, per-meta (bfb0, bfb1, a*bfb0, a*bfb1)
    cvec = np.zeros((D, 16), dtype=F32)
    cvec[:, 0] = att_b1.astype(F32)
    for m in range(P):
        a = float(prelu_a[m])
        bfb0 = (Wb[m][:, 0] + bb[m] + film_bias[m]).astype(F32)
        bfb1 = (Wb[m][:, 1] + bb[m] + film_bias[m]).astype(F32)
        cvec[:, 2 + 4 * m] = bfb0
        cvec[:, 3 + 4 * m] = bfb1
        cvec[:, 4 + 4 * m] = a * bfb0
        cvec[:, 5 + 4 * m] = a * bfb1

    # cvec16 (bf16) [128, 2]: att_w2 (col 0)
    cvec16 = np.zeros((D, 2), dtype=BF16)
    cvec16[:, 0] = att_w2.astype(BF16)
    return wmats, cvec, cvec16


# ---------------------------------------------------------------------------
# device program
# ---------------------------------------------------------------------------

def _build_program(cfg, alphas, reps=1, ablate=(), queues=4, single_packet=False,
                   scratch=16384, gbufs=3):
    _ensure_path()
    import concourse.bass as bass  # noqa: F401
    import concourse.tile as tile
    from concourse import bacc, mybir

    P = cfg["P"]
    NCOL = cfg["NCOL"]
    NBANK = cfg["NBANK"]
    pieces = cfg["pieces"]
    nch_pad = cfg["nch_pad"]
    nreg = cfg["nreg"]
    N = cfg["N"]
    NLO = cfg["NLO"]
    dt = mybir.dt
    f32 = dt.float32
    bf16 = dt.bfloat16

    nc = bacc.Bacc(
        "TRN2",
        target_bir_lowering=False,
        debug=False,
        enable_asserts=False,
        num_devices=N_CORES,
        num_swdge_queues=queues,
        dynamic_dma_scratch_size=scratch,
    )
    batch_count = [0]

    h_tab = nc.dram_tensor("h_tab", [N, D], bf16, kind="ExternalInput").ap()
    hT = nc.dram_tensor("hT", [D, NCOL], bf16, kind="ExternalInput").ap()
    blobd = {}
    for hf, tag in ((0, "L"), (1, "H")):
        nb = max(nch_pad[hf] // KG, 1)
        blobd[hf] = nc.dram_tensor(f"blob{tag}", [CHUNK, nb * MCOL], dt.int16,
                                   kind="ExternalInput").ap()
    wmatsd = nc.dram_tensor("wmats", [D, (3 * P + 1) * D], bf16,
                            kind="ExternalInput").ap()
    cvecd = nc.dram_tensor("cvec", [D, 16], f32, kind="ExternalInput").ap()
    cvec16d = nc.dram_tensor("cvec16", [D, 2], bf16,
                             kind="ExternalInput").ap()
    outd = nc.dram_tensor("outT", [D, NCOL], bf16, kind="ExternalOutput").ap()

    half_tab = {0: h_tab[0:NLO, :], 1: h_tab[NLO:N, :]}

    with tile.TileContext(nc) as tc, tc.tile_pool(name="const", bufs=1) as cpool, \
            tc.tile_pool(name="gpool", bufs=gbufs) as gpool, \
            tc.tile_pool(name="spool", bufs=gbufs) as spool, \
            tc.tile_pool(name="mpool", bufs=gbufs) as mpool, \
            tc.tile_pool(name="work", bufs=2) as work, \
            tc.tile_pool(name="ps_agg", bufs=3, space="PSUM") as ps_agg, \
            tc.tile_pool(name="ps_misc", bufs=2, space="PSUM") as ps_misc, \
            tc.tile_pool(name="ps_attn", bufs=2, space="PSUM") as ps_attn:
      for _rep in range(reps):
        # ---- constants / resident inputs ----
        hT_t = cpool.tile([D, NCOL], bf16, tag="hT", name="hT")
        nc.sync.dma_start(out=hT_t[:], in_=hT)
        wm_t = cpool.tile([D, (3 * P + 1) * D], bf16, tag="wm", name="wm")
        nc.sync.dma_start(out=wm_t[:], in_=wmatsd)
        cv_t = cpool.tile([D, 16], f32, tag="cv", name="cv")
        nc.sync.dma_start(out=cv_t[:], in_=cvecd)
        cv16_t = cpool.tile([D, 2], bf16, tag="cv16", name="cv16")
        nc.sync.dma_start(out=cv16_t[:], in_=cvec16d)
        ones_t = cpool.tile([65, D], bf16, tag="ones", name="ones")
        nc.vector.memset(ones_t[:], 1.0)

        def wmat(i):  # [128,128] lhsT block i
            return wm_t[:, i * D:(i + 1) * D]

        attW1T = wmat(3 * P)
        w2c = cv16_t[:, 0:1]
        b1c = cv_t[:, 0:1]

        # z embeddings, SBUF-resident (bf16), one tile per metapath
        z_t = [cpool.tile([D, NCOL], bf16, tag=f"z{m}", name=f"z{m}")
               for m in range(P)]
        # partitions 0/32/64 hold score rows s_m (f32)
        rows_t = cpool.tile([65, NCOL], f32, tag="rows", name="rows")
        # partitions 0/32/64 hold softmaxed beta_m (bf16)
        rows16_t = cpool.tile([65, NCOL], bf16, tag="rows16", name="rows16")

        # ---- gather + S build, two half streams ----
        # (no pre-zero needed: every chunk consumed by a matmul is fully
        # gather-written — in-plan pad edges use idx=0; only trailing
        # rounding chunks are unwritten and they are never read)
        gtiles = {}
        stiles = {}
        onehot_op = _get_seg_onehot()

        def ensure_batch(hf, g):
            if (hf, g) in gtiles:
                return
            tg = "gL" if hf == 0 else "gH"
            mb = mpool.tile([CHUNK, MCOL], dt.int16, tag="mb" + tg,
                            name="mb" + tg)
            nc.sync.dma_start(out=mb[:],
                              in_=blobd[hf][:, g * MCOL:(g + 1) * MCOL])
            ix = mb[:, 0:KG * 8]
            menc = mb[:, KG * 8:MCOL].bitcast(f32)   # [CHUNK, KG]
            gt = gpool.tile([CHUNK, KG * D], bf16, tag=tg, name=tg)
            if "nogather" not in ablate:
                nc.gpsimd.dma_gather(
                    out_ap=gt[:].rearrange("p (k d) -> p k d", k=KG),
                    in_ap=half_tab[hf],
                    idxs_ap=ix,
                    num_idxs=KG * CHUNK,
                    num_idxs_reg=int(nreg[hf][g]),
                    elem_size=D,
                    single_packet=single_packet,
                    queue_num=batch_count[0] % queues,
                )
            else:
                nc.vector.memset(gt[:, 0:1], 0.0)  # cheap writer stub
            batch_count[0] += 1
            st = spool.tile([CHUNK, KG * WIN], bf16, tag="s" + tg,
                            name="s" + tg)
            if "nos" not in ablate:
                nc.vector._custom_dve(
                    onehot_op,
                    out=st[:].rearrange("p (k w) -> p k w", k=KG),
                    in0=menc.unsqueeze(2).to_broadcast([CHUNK, KG, WIN]),
                    imm2=float(WIN),
                )
            gtiles[(hf, g)] = gt
            stiles[(hf, g)] = st

        WPB = BANK // WIN  # windows per bank
        pending_tail = [None]

        def flush_tail():
            if pending_tail[0] is not None:
                pending_tail[0]()
                pending_tail[0] = None

        def make_tail(m, b, aggh):
            def tail():
                # z_pre^T = W_t . aggh + W . hT   (accumulated in PSUM)
                fps = ps_misc.tile([D, BANK], f32, space="PSUM", tag="fps", name="fps")
                wsel = 3 * m + (0 if b < cfg["B0"] // BANK else 1)
                csl = slice(b * BANK, (b + 1) * BANK)
                nc.tensor.matmul(out=fps[:], lhsT=wmat(wsel), rhs=aggh[:],
                                 start=True, stop=False)
                nc.tensor.matmul(out=fps[:], lhsT=wmat(3 * m + 2),
                                 rhs=hT_t[:, csl], start=False, stop=True)
                # PReLU(u + bfb) = max(u + bfb, a*u + a*bfb)
                ty = 0 if b < cfg["B0"] // BANK else 1
                bfb = cv_t[:, 2 + 4 * m + ty:3 + 4 * m + ty]
                abfb = cv_t[:, 4 + 4 * m + ty:5 + 4 * m + ty]
                t0 = work.tile([D, BANK], bf16, tag="t0", name="t0")
                t1 = work.tile([D, BANK], bf16, tag="t1", name="t1")
                nc.scalar.activation(t0[:], fps[:],
                                     mybir.ActivationFunctionType.Identity,
                                     bias=bfb, scale=1.0)
                nc.scalar.activation(t1[:], fps[:],
                                     mybir.ActivationFunctionType.Identity,
                                     bias=abfb, scale=float(alphas[m]))
                nc.vector.tensor_tensor(out=z_t[m][:, csl], in0=t0[:],
                                        in1=t1[:], op=mybir.AluOpType.max)
                # attention scores for this bank
                aps = ps_attn.tile([D, BANK], f32, space="PSUM", tag="at", name="at")
                nc.tensor.matmul(out=aps[:], lhsT=attW1T, rhs=z_t[m][:, csl],
                                 start=True, stop=True)
                th = work.tile([D, BANK], bf16, tag="tanh", name="tanh")
                nc.scalar.activation(th[:], aps[:],
                                     mybir.ActivationFunctionType.Tanh,
                                     bias=b1c, scale=1.0)
                sps = ps_attn.tile([1, BANK], f32, space="PSUM", tag="at", name="at")
                nc.tensor.matmul(out=sps[:], lhsT=w2c, rhs=th[:],
                                 start=True, stop=True)
                nc.scalar.copy(out=rows_t[32 * m:32 * m + 1, csl], in_=sps[:])
            return tail

        for m in range(P):
            for b in range(NBANK):
                agg = ps_agg.tile([D, BANK], f32, space="PSUM", tag="agg", name="agg")
                for wl in range(WPB):
                    w = b * WPB + wl
                    pl = pieces[m][w]
                    for j, (hf, ch, p0, p1) in enumerate(pl):
                        g, cl = divmod(ch, KG)
                        ensure_batch(hf, g)
                        if "nomm" not in ablate:
                            nc.tensor.matmul(
                                out=agg[:, wl * WIN:(wl + 1) * WIN],
                                lhsT=gtiles[(hf, g)][p0:p1, cl * D:(cl + 1) * D],
                                rhs=stiles[(hf, g)][p0:p1,
                                                    cl * WIN:(cl + 1) * WIN],
                                start=(j == 0),
                                stop=(j == len(pl) - 1),
                            )
                # previous bank's tail: its inputs are ready by now, so the
                # in-order engines never stall on fresh PSUM/Act results
                flush_tail()
                if "nomm" in ablate:
                    if m == P - 1:
                        csl2 = slice(b * BANK, (b + 1) * BANK)
                        nc.sync.dma_start(out=outd[:, csl2], in_=hT_t[:, csl2])
                    continue
                # evacuate A@h bank to SBUF as bf16 (scalar engine copy)
                aggh = work.tile([D, BANK], bf16, tag="aggh", name="aggh")
                nc.scalar.copy(out=aggh[:], in_=agg[:])
                if "notail" in ablate:
                    if m == P - 1:
                        nc.sync.dma_start(out=outd[:, slice(b * BANK, (b + 1) * BANK)],
                                          in_=aggh[:])
                    continue
                pending_tail[0] = make_tail(m, b, aggh)
        flush_tail()

        if "notail" in ablate or "nomm" in ablate:
            continue

        # ---- softmax over metapaths (node-major [128, NCOL/128]) ----
        NMW = NCOL // D
        s_nm = [work.tile([D, NMW], f32, tag=f"snm{m}", name=f"snm{m}",
                          bufs=1) for m in range(P)]
        for m in range(P):
            nc.sync.dma_start(out=s_nm[m][:], in_=rows_t[32 * m:32 * m + 1, :])
        mx = work.tile([D, NMW], f32, tag="mx", name="mx")
        nc.vector.tensor_tensor(out=mx[:], in0=s_nm[0][:], in1=s_nm[1][:],
                                op=mybir.AluOpType.max)
        nc.vector.tensor_tensor(out=mx[:], in0=mx[:], in1=s_nm[2][:],
                                op=mybir.AluOpType.max)
        ex = [work.tile([D, NMW], f32, tag=f"ex{m}", name=f"ex{m}", bufs=1)
              for m in range(P)]
        for m in range(P):
            d = work.tile([D, NMW], f32, tag="sd", name="sd")
            nc.vector.tensor_tensor(out=d[:], in0=s_nm[m][:], in1=mx[:],
                                    op=mybir.AluOpType.subtract)
            nc.scalar.activation(ex[m][:], d[:],
                                 mybir.ActivationFunctionType.Exp)
        sm = work.tile([D, NMW], f32, tag="sm", name="sm")
        nc.vector.tensor_tensor(out=sm[:], in0=ex[0][:], in1=ex[1][:],
                                op=mybir.AluOpType.add)
        nc.vector.tensor_tensor(out=sm[:], in0=sm[:], in1=ex[2][:],
                                op=mybir.AluOpType.add)
        rc = work.tile([D, NMW], f32, tag="rc", name="rc")
        nc.vector.reciprocal(out=rc[:], in_=sm[:])
        for m in range(P):
            bt = work.tile([D, NMW], bf16, tag="bt", name="bt")
            nc.vector.tensor_tensor(out=bt[:], in0=ex[m][:], in1=rc[:],
                                    op=mybir.AluOpType.mult)
            nc.sync.dma_start(out=rows16_t[32 * m:32 * m + 1, :], in_=bt[:])

        # ---- final combine per bank: out = sum_m beta_m * z_m  (+h on host)
        for b in range(NBANK):
            csl = slice(b * BANK, (b + 1) * BANK)
            acc = work.tile([D, BANK], bf16, tag="acc", name="acc")
            tmp = work.tile([D, BANK], bf16, tag="tmp", name="tmp")
            for m in range(P):
                bps = ps_misc.tile([D, BANK], f32, space="PSUM", tag="fps", name="fps")
                nc.tensor.matmul(out=bps[:], lhsT=ones_t[32 * m:32 * m + 1, :],
                                 rhs=rows16_t[32 * m:32 * m + 1, csl],
                                 start=True, stop=True)
                bc16 = work.tile([D, BANK], bf16, tag="bc16", name="bc16")
                nc.scalar.copy(out=bc16[:], in_=bps[:])
                dst = acc if m == 0 else tmp
                nc.vector.tensor_tensor(out=dst[:], in0=z_t[m][:, csl],
                                        in1=bc16[:], op=mybir.AluOpType.mult)
                if m > 0:
                    nc.vector.tensor_tensor(out=acc[:], in0=acc[:],
                                            in1=tmp[:],
                                            op=mybir.AluOpType.add)
            nc.sync.dma_start(out=outd[:, csl], in_=acc[:])

    nc.compile()
    return nc


# ---------------------------------------------------------------------------
# benchmarking (test-only; not used by the grading path)
# ---------------------------------------------------------------------------

def _make_runner(nc, in_maps):
    """Build a jitted runner for a prebuilt program with device-resident
    inputs; returns (fn, dev_args) where fn(*dev_args) executes once."""
    import jax
    from jax.experimental.shard_map import shard_map
    from jax.sharding import Mesh, PartitionSpec

    from concourse import mybir
    from concourse.bass2jax import (
        _bass_exec_p,
        install_neuronx_cc_hook,
        partition_id_tensor,
    )

    install_neuronx_cc_hook()
    n_cores = len(in_maps)
    partition_name = (
        nc.partition_id_tensor.name if nc.partition_id_tensor else None
    )
    in_names, out_names, out_avals, zero_outs = [], [], [], []
    for alloc in nc.m.functions[0].allocations:
        if not isinstance(alloc, mybir.MemoryLocationSet):
            continue
        name = alloc.memorylocations[0].name
        if alloc.kind == "ExternalInput":
            if name != partition_name:
                in_names.append(name)
        elif alloc.kind == "ExternalOutput":
            out_names.append(name)
            shape = tuple(alloc.tensor_shape)
            dtype = mybir.dt.np(alloc.dtype)
            out_avals.append(jax.core.ShapedArray(shape, dtype))
            zero_outs.append(np.zeros(shape, dtype))
    n_params = len(in_names)
    bind_names = tuple(
        in_names + out_names + ([partition_name] if partition_name else [])
    )

    def _body(*args):
        operands = list(args)
        if partition_name is not None:
            operands.append(partition_id_tensor())
        outs = _bass_exec_p.bind(
            *operands,
            out_avals=tuple(out_avals),
            in_names=bind_names,
            out_names=tuple(out_names),
            lowering_input_output_aliases=(),
            sim_require_finite=True,
            sim_require_nnan=True,
            nc=nc,
        )
        return tuple(outs)

    devices = jax.devices()[:n_cores]
    mesh = Mesh(np.asarray(devices), ("core",))
    fn = jax.jit(
        shard_map(
            _body,
            mesh=mesh,
            in_specs=(PartitionSpec("core"),) * (n_params + len(out_names)),
            out_specs=(PartitionSpec("core"),) * len(out_names),
            check_rep=False,
        )
    )
    concat_in = [
        np.concatenate([np.asarray(m[name]) for m in in_maps], axis=0)
        for name in in_names
    ]
    concat_zero = [
        np.zeros((n_cores * z.shape[0], *z.shape[1:]), z.dtype) for z in zero_outs
    ]
    dev_args = [jax.device_put(a) for a in concat_in + concat_zero]
    return fn, dev_args


def bench_exec_ns(cfg, alphas, in_maps, nc1, reps_hi=41, timing_reps=7):
    """Difference wall time of a 1-rep vs reps_hi-rep program (kernel body
    repeated inside one NEFF) to cancel the dispatch round trip."""
    import time

    import jax

    nc_hi = _build_program(cfg, alphas, reps=reps_hi)
    results = {}
    for label, nc in (("lo", nc1), ("hi", nc_hi)):
        fn, dev_args = _make_runner(nc, in_maps)
        jax.block_until_ready(fn(*dev_args))  # compile + warm
        best = float("inf")
        for _ in range(timing_reps):
            t0 = time.perf_counter()
            jax.block_until_ready(fn(*dev_args))
            best = min(best, time.perf_counter() - t0)
        results[label] = best
    exec_ns = (results["hi"] - results["lo"]) / (reps_hi - 1) * 1e9
    return exec_ns, results


# ---------------------------------------------------------------------------
# entry point
# ---------------------------------------------------------------------------

def kernel(h, edge_rows, edge_cols, edge_vals, node_type,
           W_fc, prelu_a, Wg, bg, Wb, bb, film_bias,
           att_W1, att_b1, att_w2, _run_opts=None):
    _ensure_path()
    from concourse import bass_utils

    h = np.asarray(h, dtype=F32)
    edge_rows = np.asarray(edge_rows)
    edge_cols = np.asarray(edge_cols)
    edge_vals = np.asarray(edge_vals, dtype=F32)
    node_type = np.asarray(node_type)

    cfg, per_core = _plan(h, edge_rows, edge_cols, edge_vals, node_type)
    wmats, cvec, cvec16 = _pack_weights(
        cfg, np.asarray(W_fc), np.asarray(prelu_a),
        np.asarray(Wg), np.asarray(bg),
        np.asarray(Wb), np.asarray(bb),
        np.asarray(film_bias), np.asarray(att_W1),
        np.asarray(att_b1), np.asarray(att_w2))

    nc = _build_program(cfg, np.asarray(prelu_a, dtype=F32))

    npc = cfg["npc"]
    B0 = cfg["B0"]
    NCOL = cfg["NCOL"]
    h16 = h.astype(BF16)
    in_maps = []
    for c in range(N_CORES):
        pc = per_core[c]
        hT_own = np.zeros((D, NCOL), dtype=BF16)
        own = h16[c * npc:(c + 1) * npc]      # [npc, D]
        srt = own[pc["perm"]]                 # type-sorted rows
        n0 = pc["n0"]
        hT_own[:, :n0] = srt[:n0].T
        hT_own[:, B0:B0 + (npc - n0)] = srt[n0:].T
        im = {
            "h_tab": h16,
            "hT": hT_own,
            "wmats": wmats,
            "cvec": cvec,
            "cvec16": cvec16,
        }
        for tag in ("L", "H"):
            arr = pc["blob" + tag]
            if arr.shape[1] == 0:  # empty stream: dram tensor padded
                arr = np.full((CHUNK, MCOL), -1, dtype=np.int16)
            im["blob" + tag] = arr
        in_maps.append(im)

    run_kwargs = dict(_run_opts or {})
    bench = run_kwargs.pop("bench", None)
    run_kwargs.pop("_result", None)
    run_kwargs.pop("_bench_ns", None)
    run_kwargs.pop("_bench_times", None)
    res = bass_utils.run_bass_kernel_spmd(
        nc, in_maps, core_ids=list(range(N_CORES)), **run_kwargs
    )

    out = np.empty((cfg["N"], D), dtype=F32)
    for c in range(N_CORES):
        pc = per_core[c]
        n0 = pc["n0"]
        zT = res.results[c]["outT"].astype(F32)   # [D, NCOL] bf16 -> f32
        real = np.concatenate(
            [zT[:, :n0], zT[:, B0:B0 + (npc - n0)]], axis=1
        ).T                                    # [npc, D] sorted order
        shard = np.empty((npc, D), dtype=F32)
        shard[pc["perm"]] = real
        out[c * npc:(c + 1) * npc] = shard + h[c * npc:(c + 1) * npc]

    if bench:
        exec_ns, times = bench_exec_ns(
            cfg, np.asarray(prelu_a, dtype=F32), in_maps, nc)
        if isinstance(_run_opts, dict):
            _run_opts["_bench_ns"] = exec_ns
            _run_opts["_bench_times"] = times
    if isinstance(_run_opts, dict):
        _run_opts["_result"] = res
    return out


# revision 56
# speedup vs baseline: 369.2073x; 3.8298x over previous
"""MGNN (gnn_message_passing) Trainium2 kernel.

Strategy (8 NeuronCores, destination-sharded, no collectives):
  - Each core owns N/8 = 6250 destination nodes. Host partitions the edge
    lists by destination row, sorts by (local) destination, and pads edge
    chunks so all 8 cores run an identical SPMD program.
  - Aggregation identity: agg_i = segsum(val * (h @ W_i^T)[col])
                                = segsum(val * h[col]) @ W_i^T
    so the gather table is h itself for all 3 metapaths; the per-metapath
    weight matmul is applied after aggregation (on [D, n] tiles).
  - On device (feature-major layout [D=128 partitions, nodes on free dim]):
      * batched indirect-DMA gathers of h rows (bf16, 128 rows/chunk, KG
        chunks per DMA instruction); out-of-range pad indices are skipped.
      * segment-sum via one-hot matmul (bf16): S[e, d] = val_e *
        (iota[d]==doff_e), PSUM accumulation per 64-destination window.
      * FiLM folded into weights: gamma handled by sorting each core's
        nodes by type (host) and using two pre-scaled weight matrices.
        seq_fts residual is matmul-accumulated into the same PSUM tile.
      * PReLU(u) = max(u + bfb, a*u + a*bfb) via two scalar-engine affine
        ops + vector max; z kept resident in SBUF (bf16).
      * Semantics attention: tanh/score matmuls in feature-major form,
        softmax computed node-major after an SBUF reshape DMA.
  - Output is written feature-major bf16 [128, NCOL]; host transposes,
    strips padding, undoes the type-sort permutation, adds the +h residual
    in fp32 and concatenates shards.
"""

import math
import os

import numpy as np
import ml_dtypes

BF16 = ml_dtypes.bfloat16


def _ensure_path():
    try:
        import concourse  # noqa: F401
    except ImportError:
        import sys

        for p in ("/opt/trn_rl_repo", "/root/.axon_site/_ro/trn_rl_repo"):
            if os.path.isdir(p) and p not in sys.path:
                sys.path.insert(0, p)


# ---------------------------------------------------------------------------
# configuration
# ---------------------------------------------------------------------------

N_CORES = 8
D = 128          # hidden dim (= partition count)
CHUNK = 128      # edges per matmul chunk (contraction dim)
WIN = 64         # destinations per one-hot window (S width)
KG = 32          # chunks per dma_gather instruction
BANK = 512      # psum bank width (f32 elems) = 8 windows
PAD_COL = 1 << 28  # out-of-bounds gather index for pad edges (skipped)

F32 = np.float32
I32 = np.int32

# meta blob layout (int16 cols per gather batch): idx wrap + enc(f32) bits
MCOL = KG * 8 + KG * 2  # per-batch int16 columns: [idx | enc]


# ---------------------------------------------------------------------------
# custom DVE op: fused one-hot S build
#   S[p, k, w] = relu(d) * (d < 1),  d = enc[p, k] - w
#   where enc = doff + val packs the window offset (integer) and the edge
#   value (fraction) into one fp32; w is recovered on-engine from the
#   element position (Idx - PageIdx(0, WIN)).
# ---------------------------------------------------------------------------

_SEG_ONEHOT = None


def _get_seg_onehot():
    global _SEG_ONEHOT
    if _SEG_ONEHOT is not None:
        return _SEG_ONEHOT
    _ensure_path()
    from concourse import dve_ops
    from concourse.dve_ops import (
        _SUB_OPCODE_FOR_NAME,
        OPS,
        DveOp,
        has_src1,
        lower,
    )
    from concourse.dve_spec import C2, Idx, One, PageIdx, Spec, Src0, Zero, relu
    from concourse.dve_uop import DveOpSpec

    name = "SEG_ONEHOT_MGNN"
    if name in _SUB_OPCODE_FOR_NAME:
        _SEG_ONEHOT = next(o for o in OPS if o.name == name)
        return _SEG_ONEHOT

    d = Src0 - (Idx - PageIdx(Zero, C2))

    def _ref(in0, in1, s0, s1, imm2):
        in0 = np.asarray(in0, dtype=np.float32)
        S, N = in0.shape[-2], in0.shape[-1]
        idx = np.arange(S * N, dtype=np.float32).reshape(S, N)
        pg = (np.arange(S, dtype=np.float32) * imm2)[:, None]
        dd = in0 - (idx - pg)
        return (np.maximum(dd, 0.0) * (dd < 1.0)).astype(np.float32)

    spec = Spec(body=relu(d) * (d < One), reference=_ref)
    row = max(_SUB_OPCODE_FOR_NAME.values()) + 1
    assert row < 0x20
    shas = {}
    for ver in ("v3", "v4"):
        try:
            tmp = DveOpSpec(name=name, opcode=row, uops=lower(spec, ver=ver),
                            rd1_en=has_src1(spec))
            shas[ver] = tmp.sha(ver)
        except Exception:
            pass
    assert shas, "SEG_ONEHOT spec failed to lower"
    op = DveOp(name, spec, subdim=True, uops_sha=shas)
    _SUB_OPCODE_FOR_NAME[name] = row
    OPS.append(op)
    dve_ops.CUSTOM_DVE_SPECS[name] = spec
    _SEG_ONEHOT = op
    return op


# ---------------------------------------------------------------------------
# host-side planning
# ---------------------------------------------------------------------------

def _round_up(x, m):
    return (x + m - 1) // m * m


def _round_up_arr(a, m):
    return (a + m - 1) // m * m


def _plan(h, edge_rows, edge_cols, edge_vals, node_type):
    """Shard by destination, type-sort each shard, build padded chunk plan.

    Chunks are segregated by source half (dma_gather indices are int16, so
    the gather table is split at NLO = N//2). Returns (cfg, per_core).
    """
    N = h.shape[0]
    P = edge_rows.shape[0]
    npc = N // N_CORES
    assert npc * N_CORES == N
    NLO = N // 2
    assert NLO <= 32768 and (N - NLO) <= 32768

    # --- per-core destination shards, sorted by node_type (stable) ---
    shards = []
    for c in range(N_CORES):
        own = slice(c * npc, (c + 1) * npc)
        t = node_type[own]
        perm = np.argsort(t, kind="stable")  # sorted-rank -> original local id
        n0 = int((t == 0).sum())
        shards.append({"perm": perm, "n0": n0})

    max_n0 = max(s["n0"] for s in shards)
    max_n1 = max(npc - s["n0"] for s in shards)
    B0 = _round_up(max(max_n0, 1), BANK)
    NCOL = B0 + _round_up(max(max_n1, 1), BANK)
    NBANK = NCOL // BANK
    NWIN = NCOL // WIN

    # padded-column map per core: local node id -> column
    for s in shards:
        inv = np.empty(npc, dtype=np.int64)
        inv[s["perm"]] = np.arange(npc)  # original local id -> sorted rank
        col = np.where(inv < s["n0"], inv, B0 + (inv - s["n0"]))
        s["colmap"] = col

    # --- edge bucketing by (core, metapath, half, window) ---
    edge_data = [[None] * P for _ in range(N_CORES)]
    hist = np.zeros((2, N_CORES, P, NWIN), dtype=np.int64)
    for c in range(N_CORES):
        base = c * npc
        for m in range(P):
            er = edge_rows[m]
            mask = (er >= base) & (er < base + npc)
            dl = shards[c]["colmap"][er[mask] - base]
            cs = edge_cols[m][mask].astype(np.int64)
            vs = edge_vals[m][mask].astype(F32)
            half = (cs >= NLO).astype(np.int64)
            # sort by (window, half) so each (w, half) group is contiguous
            key = (dl // WIN) * 2 + half
            order = np.argsort(key, kind="stable")
            dl = dl[order]
            cs = cs[order]
            vs = vs[order]
            half = half[order]
            w = dl // WIN
            for hf in range(2):
                hist[hf, c, m] += np.bincount(w[half == hf], minlength=NWIN)
            edge_data[c][m] = (dl, cs, vs, w, half)

    QNT = 128  # group padding quantum (multiple of 32; 128 = whole chunks)
    gl = np.maximum(QNT, _round_up_arr(hist[0].max(axis=0), QNT))  # [P, NWIN]
    gh = _round_up_arr(hist[1].max(axis=0), QNT)
    gsz = np.stack([gl, gh])               # [2, P, NWIN] padded slot counts

    # slot base per (half, m, w); matmul base partition must be 0/32/64, so
    # lead-pad any nonzero group that would start at slot 96 (mod 128)
    base_slot = np.zeros((2, P, NWIN), dtype=np.int64)
    tot_slots = [0, 0]
    for hf in range(2):
        flat = gsz[hf].reshape(-1)
        bflat = base_slot[hf].reshape(-1)
        base = 0
        for i in range(flat.size):
            if flat[i] > 0 and base % CHUNK == 96:
                base += 32
            bflat[i] = base
            base += int(flat[i])
        tot_slots[hf] = base
    nch_pad = [_round_up(-(-t // CHUNK), KG) for t in tot_slots]

    # matmul pieces per (m, w): (hf, chunk, p0, p1) — 32-aligned partition
    # sub-ranges of gather chunks, in stream order
    pieces = [[[] for _ in range(NWIN)] for _ in range(P)]
    for m in range(P):
        for w in range(NWIN):
            for hf in range(2):
                s0 = int(base_slot[hf, m, w])
                s1 = s0 + int(gsz[hf, m, w])
                s = s0
                while s < s1:
                    p0 = s % CHUNK
                    # PE operand partition ranges: base 0 -> <=128,
                    # base 32 -> <=32, base 64 -> <=64 (base 96 illegal)
                    cap = 128 if p0 == 0 else (32 if p0 == 32 else 64)
                    e = min(s1, s + cap)
                    pieces[m][w].append((hf, s // CHUNK, p0, p0 + (e - s)))
                    s = e

    per_core = []
    for c in range(N_CORES):
        streams = []
        for hf in range(2):
            nitems = nch_pad[hf] * CHUNK
            streams.append({
                "idx": np.full(nitems, -1, dtype=np.int64),
                "enc": np.zeros(nitems, dtype=F32),
            })
            # in-plan slots: pad items default to row 0 / enc 0
            streams[hf]["idx"][:tot_slots[hf]] = 0
        for m in range(P):
            dl, cs, vs, w, half = edge_data[c][m]
            for hf in range(2):
                sel = half == hf
                wm_ = w[sel]
                starts = np.searchsorted(wm_, np.arange(NWIN))
                rank = np.arange(len(wm_)) - starts[wm_]
                slot = base_slot[hf, m, wm_] + rank
                st = streams[hf]
                st["idx"][slot] = cs[sel] - hf * NLO
                # enc packs the window offset (int) + edge value (frac)
                st["enc"][slot] = (dl[sel] - wm_ * WIN).astype(F32) + vs[sel]
        pc = {"perm": shards[c]["perm"], "n0": shards[c]["n0"]}
        for hf, tag in ((0, "L"), (1, "H")):
            st = streams[hf]
            nb = nch_pad[hf] // KG
            # idx items wrapped in 16 partitions, replicated to 128
            iw = np.ascontiguousarray(
                st["idx"].reshape(-1, 16).T).astype(np.int16)   # [16, items/16]
            iw = np.tile(iw, (8, 1))                            # [128, items/16]
            enc = np.ascontiguousarray(
                st["enc"].reshape(-1, CHUNK).T)                 # [128, nch_pad]
            enc16 = enc.view(np.int16)                          # [128, 2*nch_pad]
            blob = np.empty((CHUNK, nb * MCOL), dtype=np.int16)
            for g in range(nb):
                blob[:, g * MCOL:g * MCOL + KG * 8] = \
                    iw[:, g * KG * 8:(g + 1) * KG * 8]
                blob[:, g * MCOL + KG * 8:(g + 1) * MCOL] = \
                    enc16[:, g * 2 * KG:(g + 1) * 2 * KG]
            pc["blob" + tag] = blob
        per_core.append(pc)

    # per-gather-batch valid index counts (same for all cores by construction)
    nreg = []
    for hf in range(2):
        nb = nch_pad[hf] // KG
        r = []
        for g in range(nb):
            r.append(max(0, min(tot_slots[hf] - g * KG * CHUNK, KG * CHUNK)))
        nreg.append(r)

    cfg = {
        "N": N,
        "NLO": NLO,
        "P": P,
        "npc": npc,
        "B0": B0,
        "NCOL": NCOL,
        "NBANK": NBANK,
        "NWIN": NWIN,
        "pieces": pieces,
        "tot_slots": tot_slots,
        "nch_pad": nch_pad,
        "nreg": nreg,
    }
    return cfg, per_core


def _pack_weights(cfg, W_fc, prelu_a, Wg, bg, Wb, bb, film_bias,
                  att_W1, att_b1, att_w2):
    """Pack small weights into dense blobs (replicated to every core)."""
    P = cfg["P"]
    # wmats: per meta [W0T, W1T, WfcT], then att_W1T  -> [128, (3P+1)*128]
    blocks = []
    for m in range(P):
        g0 = (Wg[m][:, 0] + bg[m]).astype(F32)  # [D]
        g1 = (Wg[m][:, 1] + bg[m]).astype(F32)
        WT = W_fc[m].T.astype(F32)              # [fi, fo]
        blocks += [WT * g0[None, :], WT * g1[None, :], WT]
    blocks.append(att_W1.T.astype(F32))          # lhsT[d, hid]
    wmats = np.ascontiguousarray(
        np.concatenate(blocks, axis=1)).astype(BF16)

    # cvec (f32) [128, 16]: b1, per-meta (bfb0, bfb1, a*bfb0, a*bfb1)
    cvec = np.zeros((D, 16), dtype=F32)
    cvec[:, 0] = att_b1.astype(F32)
    for m in range(P):
        a = float(prelu_a[m])
        bfb0 = (Wb[m][:, 0] + bb[m] + film_bias[m]).astype(F32)
        bfb1 = (Wb[m][:, 1] + bb[m] + film_bias[m]).astype(F32)
        cvec[:, 2 + 4 * m] = bfb0
        cvec[:, 3 + 4 * m] = bfb1
        cvec[:, 4 + 4 * m] = a * bfb0
        cvec[:, 5 + 4 * m] = a * bfb1

    # cvec16 (bf16) [128, 2]: att_w2 (col 0)
    cvec16 = np.zeros((D, 2), dtype=BF16)
    cvec16[:, 0] = att_w2.astype(BF16)
    return wmats, cvec, cvec16


# ---------------------------------------------------------------------------
# device program
# ---------------------------------------------------------------------------

def _build_program(cfg, alphas, reps=1, ablate=(), queues=4, single_packet=False,
                   scratch=16384, gbufs=3):
    _ensure_path()
    import concourse.bass as bass  # noqa: F401
    import concourse.tile as tile
    from concourse import bacc, mybir

    P = cfg["P"]
    NCOL = cfg["NCOL"]
    NBANK = cfg["NBANK"]
    pieces = cfg["pieces"]
    nch_pad = cfg["nch_pad"]
    nreg = cfg["nreg"]
    N = cfg["N"]
    NLO = cfg["NLO"]
    dt = mybir.dt
    f32 = dt.float32
    bf16 = dt.bfloat16

    nc = bacc.Bacc(
        "TRN2",
        target_bir_lowering=False,
        debug=False,
        enable_asserts=False,
        num_devices=N_CORES,
        num_swdge_queues=queues,
        dynamic_dma_scratch_size=scratch,
    )
    batch_count = [0]

    h_tab = nc.dram_tensor("h_tab", [N, D], bf16, kind="ExternalInput").ap()
    hT = nc.dram_tensor("hT", [D, NCOL], bf16, kind="ExternalInput").ap()
    blobd = {}
    for hf, tag in ((0, "L"), (1, "H")):
        nb = max(nch_pad[hf] // KG, 1)
        blobd[hf] = nc.dram_tensor(f"blob{tag}", [CHUNK, nb * MCOL], dt.int16,
                                   kind="ExternalInput").ap()
    wmatsd = nc.dram_tensor("wmats", [D, (3 * P + 1) * D], bf16,
                            kind="ExternalInput").ap()
    cvecd = nc.dram_tensor("cvec", [D, 16], f32, kind="ExternalInput").ap()
    cvec16d = nc.dram_tensor("cvec16", [D, 2], bf16,
                             kind="ExternalInput").ap()
    outd = nc.dram_tensor("outT", [D, NCOL], bf16, kind="ExternalOutput").ap()

    half_tab = {0: h_tab[0:NLO, :], 1: h_tab[NLO:N, :]}

    with tile.TileContext(nc) as tc, tc.tile_pool(name="const", bufs=1) as cpool, \
            tc.tile_pool(name="gpool", bufs=gbufs) as gpool, \
            tc.tile_pool(name="spool", bufs=gbufs) as spool, \
            tc.tile_pool(name="mpool", bufs=gbufs) as mpool, \
            tc.tile_pool(name="work", bufs=2) as work, \
            tc.tile_pool(name="ps_agg", bufs=3, space="PSUM") as ps_agg, \
            tc.tile_pool(name="ps_misc", bufs=2, space="PSUM") as ps_misc, \
            tc.tile_pool(name="ps_attn", bufs=2, space="PSUM") as ps_attn:
      for _rep in range(reps):
        # ---- constants / resident inputs ----
        hT_t = cpool.tile([D, NCOL], bf16, tag="hT", name="hT")
        nc.sync.dma_start(out=hT_t[:], in_=hT)
        wm_t = cpool.tile([D, (3 * P + 1) * D], bf16, tag="wm", name="wm")
        nc.sync.dma_start(out=wm_t[:], in_=wmatsd)
        cv_t = cpool.tile([D, 16], f32, tag="cv", name="cv")
        nc.sync.dma_start(out=cv_t[:], in_=cvecd)
        cv16_t = cpool.tile([D, 2], bf16, tag="cv16", name="cv16")
        nc.sync.dma_start(out=cv16_t[:], in_=cvec16d)
        ones_t = cpool.tile([65, D], bf16, tag="ones", name="ones")
        nc.vector.memset(ones_t[:], 1.0)

        def wmat(i):  # [128,128] lhsT block i
            return wm_t[:, i * D:(i + 1) * D]

        attW1T = wmat(3 * P)
        w2c = cv16_t[:, 0:1]
        b1c = cv_t[:, 0:1]

        # z embeddings, SBUF-resident (bf16), one tile per metapath
        z_t = [cpool.tile([D, NCOL], bf16, tag=f"z{m}", name=f"z{m}")
               for m in range(P)]
        # partitions 0/32/64 hold score rows s_m (f32)
        rows_t = cpool.tile([65, NCOL], f32, tag="rows", name="rows")
        # partitions 0/32/64 hold softmaxed beta_m (bf16)
        rows16_t = cpool.tile([65, NCOL], bf16, tag="rows16", name="rows16")

        # ---- gather + S build, two half streams ----
        # (no pre-zero needed: every chunk consumed by a matmul is fully
        # gather-written — in-plan pad edges use idx=0; only trailing
        # rounding chunks are unwritten and they are never read)
        gtiles = {}
        stiles = {}
        onehot_op = _get_seg_onehot()

        def ensure_batch(hf, g):
            if (hf, g) in gtiles:
                return
            tg = "gL" if hf == 0 else "gH"
            mb = mpool.tile([CHUNK, MCOL], dt.int16, tag="mb" + tg,
                            name="mb" + tg)
            nc.sync.dma_start(out=mb[:],
                              in_=blobd[hf][:, g * MCOL:(g + 1) * MCOL])
            ix = mb[:, 0:KG * 8]
            menc = mb[:, KG * 8:MCOL].bitcast(f32)   # [CHUNK, KG]
            gt = gpool.tile([CHUNK, KG * D], bf16, tag=tg, name=tg)
            if "nogather" not in ablate:
                nc.gpsimd.dma_gather(
                    out_ap=gt[:].rearrange("p (k d) -> p k d", k=KG),
                    in_ap=half_tab[hf],
                    idxs_ap=ix,
                    num_idxs=KG * CHUNK,
                    num_idxs_reg=int(nreg[hf][g]),
                    elem_size=D,
                    single_packet=single_packet,
                    queue_num=batch_count[0] % queues,
                )
            else:
                nc.vector.memset(gt[:, 0:1], 0.0)  # cheap writer stub
            batch_count[0] += 1
            st = spool.tile([CHUNK, KG * WIN], bf16, tag="s" + tg,
                            name="s" + tg)
            if "nos" not in ablate:
                nc.vector._custom_dve(
                    onehot_op,
                    out=st[:].rearrange("p (k w) -> p k w", k=KG),
                    in0=menc.unsqueeze(2).to_broadcast([CHUNK, KG, WIN]),
                    imm2=float(WIN),
                )
            gtiles[(hf, g)] = gt
            stiles[(hf, g)] = st

        WPB = BANK // WIN  # windows per bank
        pending_tail = [None]

        def flush_tail():
            if pending_tail[0] is not None:
                pending_tail[0]()
                pending_tail[0] = None

        def make_tail(m, b, aggh):
            def tail():
                # z_pre^T = W_t . aggh + W . hT   (accumulated in PSUM)
                fps = ps_misc.tile([D, BANK], f32, space="PSUM", tag="fps", name="fps")
                wsel = 3 * m + (0 if b < cfg["B0"] // BANK else 1)
                csl = slice(b * BANK, (b + 1) * BANK)
                nc.tensor.matmul(out=fps[:], lhsT=wmat(wsel), rhs=aggh[:],
                                 start=True, stop=False)
                nc.tensor.matmul(out=fps[:], lhsT=wmat(3 * m + 2),
                                 rhs=hT_t[:, csl], start=False, stop=True)
                # PReLU(u + bfb) = max(u + bfb, a*u + a*bfb)
                ty = 0 if b < cfg["B0"] // BANK else 1
                bfb = cv_t[:, 2 + 4 * m + ty:3 + 4 * m + ty]
                abfb = cv_t[:, 4 + 4 * m + ty:5 + 4 * m + ty]
                t0 = work.tile([D, BANK], bf16, tag="t0", name="t0")
                t1 = work.tile([D, BANK], bf16, tag="t1", name="t1")
                nc.scalar.activation(t0[:], fps[:],
                                     mybir.ActivationFunctionType.Identity,
                                     bias=bfb, scale=1.0)
                nc.scalar.activation(t1[:], fps[:],
                                     mybir.ActivationFunctionType.Identity,
                                     bias=abfb, scale=float(alphas[m]))
                nc.vector.tensor_tensor(out=z_t[m][:, csl], in0=t0[:],
                                        in1=t1[:], op=mybir.AluOpType.max)
                # attention scores for this bank
                aps = ps_attn.tile([D, BANK], f32, space="PSUM", tag="at", name="at")
                nc.tensor.matmul(out=aps[:], lhsT=attW1T, rhs=z_t[m][:, csl],
                                 start=True, stop=True)
                th = work.tile([D, BANK], bf16, tag="tanh", name="tanh")
                nc.scalar.activation(th[:], aps[:],
                                     mybir.ActivationFunctionType.Tanh,
                                     bias=b1c, scale=1.0)
                sps = ps_attn.tile([1, BANK], f32, space="PSUM", tag="at", name="at")
                nc.tensor.matmul(out=sps[:], lhsT=w2c, rhs=th[:],
                                 start=True, stop=True)
                nc.scalar.copy(out=rows_t[32 * m:32 * m + 1, csl], in_=sps[:])
            return tail

        for m in range(P):
            for b in range(NBANK):
                agg = ps_agg.tile([D, BANK], f32, space="PSUM", tag="agg", name="agg")
                for wl in range(WPB):
                    w = b * WPB + wl
                    pl = pieces[m][w]
                    for j, (hf, ch, p0, p1) in enumerate(pl):
                        g, cl = divmod(ch, KG)
                        ensure_batch(hf, g)
                        if "nomm" not in ablate:
                            nc.tensor.matmul(
                                out=agg[:, wl * WIN:(wl + 1) * WIN],
                                lhsT=gtiles[(hf, g)][p0:p1, cl * D:(cl + 1) * D],
                                rhs=stiles[(hf, g)][p0:p1,
                                                    cl * WIN:(cl + 1) * WIN],
                                start=(j == 0),
                                stop=(j == len(pl) - 1),
                            )
                # previous bank's tail: its inputs are ready by now, so the
                # in-order engines never stall on fresh PSUM/Act results
                flush_tail()
                if "nomm" in ablate:
                    if m == P - 1:
                        csl2 = slice(b * BANK, (b + 1) * BANK)
                        nc.sync.dma_start(out=outd[:, csl2], in_=hT_t[:, csl2])
                    continue
                # evacuate A@h bank to SBUF as bf16 (scalar engine copy)
                aggh = work.tile([D, BANK], bf16, tag="aggh", name="aggh")
                nc.scalar.copy(out=aggh[:], in_=agg[:])
                if "notail" in ablate:
                    if m == P - 1:
                        nc.sync.dma_start(out=outd[:, slice(b * BANK, (b + 1) * BANK)],
                                          in_=aggh[:])
                    continue
                pending_tail[0] = make_tail(m, b, aggh)
        flush_tail()

        if "notail" in ablate or "nomm" in ablate:
            continue

        # ---- softmax over metapaths (node-major [128, NCOL/128]) ----
        NMW = NCOL // D
        s_nm = [work.tile([D, NMW], f32, tag=f"snm{m}", name=f"snm{m}",
                          bufs=1) for m in range(P)]
        for m in range(P):
            nc.sync.dma_start(out=s_nm[m][:], in_=rows_t[32 * m:32 * m + 1, :])
        mx = work.tile([D, NMW], f32, tag="mx", name="mx")
        nc.vector.tensor_tensor(out=mx[:], in0=s_nm[0][:], in1=s_nm[1][:],
                                op=mybir.AluOpType.max)
        nc.vector.tensor_tensor(out=mx[:], in0=mx[:], in1=s_nm[2][:],
                                op=mybir.AluOpType.max)
        ex = [work.tile([D, NMW], f32, tag=f"ex{m}", name=f"ex{m}", bufs=1)
              for m in range(P)]
        for m in range(P):
            d = work.tile([D, NMW], f32, tag="sd", name="sd")
            nc.vector.tensor_tensor(out=d[:], in0=s_nm[m][:], in1=mx[:],
                                    op=mybir.AluOpType.subtract)
            nc.scalar.activation(ex[m][:], d[:],
                                 mybir.ActivationFunctionType.Exp)
        sm = work.tile([D, NMW], f32, tag="sm", name="sm")
        nc.vector.tensor_tensor(out=sm[:], in0=ex[0][:], in1=ex[1][:],
                                op=mybir.AluOpType.add)
        nc.vector.tensor_tensor(out=sm[:], in0=sm[:], in1=ex[2][:],
                                op=mybir.AluOpType.add)
        rc = work.tile([D, NMW], f32, tag="rc", name="rc")
        nc.vector.reciprocal(out=rc[:], in_=sm[:])
        for m in range(P):
            bt = work.tile([D, NMW], bf16, tag="bt", name="bt")
            nc.vector.tensor_tensor(out=bt[:], in0=ex[m][:], in1=rc[:],
                                    op=mybir.AluOpType.mult)
            nc.sync.dma_start(out=rows16_t[32 * m:32 * m + 1, :], in_=bt[:])

        # ---- final combine per bank: out = sum_m beta_m * z_m  (+h on host)
        for b in range(NBANK):
            csl = slice(b * BANK, (b + 1) * BANK)
            acc = work.tile([D, BANK], bf16, tag="acc", name="acc")
            tmp = work.tile([D, BANK], bf16, tag="tmp", name="tmp")
            for m in range(P):
                bps = ps_misc.tile([D, BANK], f32, space="PSUM", tag="fps", name="fps")
                nc.tensor.matmul(out=bps[:], lhsT=ones_t[32 * m:32 * m + 1, :],
                                 rhs=rows16_t[32 * m:32 * m + 1, csl],
                                 start=True, stop=True)
                bc16 = work.tile([D, BANK], bf16, tag="bc16", name="bc16")
                nc.scalar.copy(out=bc16[:], in_=bps[:])
                dst = acc if m == 0 else tmp
                nc.vector.tensor_tensor(out=dst[:], in0=z_t[m][:, csl],
                                        in1=bc16[:], op=mybir.AluOpType.mult)
                if m > 0:
                    nc.vector.tensor_tensor(out=acc[:], in0=acc[:],
                                            in1=tmp[:],
                                            op=mybir.AluOpType.add)
            nc.sync.dma_start(out=outd[:, csl], in_=acc[:])

    nc.compile()
    return nc


# ---------------------------------------------------------------------------
# benchmarking (test-only; not used by the grading path)
# ---------------------------------------------------------------------------

def _make_runner(nc, in_maps):
    """Build a jitted runner for a prebuilt program with device-resident
    inputs; returns (fn, dev_args) where fn(*dev_args) executes once."""
    import jax
    from jax.experimental.shard_map import shard_map
    from jax.sharding import Mesh, PartitionSpec

    from concourse import mybir
    from concourse.bass2jax import (
        _bass_exec_p,
        install_neuronx_cc_hook,
        partition_id_tensor,
    )

    install_neuronx_cc_hook()
    n_cores = len(in_maps)
    partition_name = (
        nc.partition_id_tensor.name if nc.partition_id_tensor else None
    )
    in_names, out_names, out_avals, zero_outs = [], [], [], []
    for alloc in nc.m.functions[0].allocations:
        if not isinstance(alloc, mybir.MemoryLocationSet):
            continue
        name = alloc.memorylocations[0].name
        if alloc.kind == "ExternalInput":
            if name != partition_name:
                in_names.append(name)
        elif alloc.kind == "ExternalOutput":
            out_names.append(name)
            shape = tuple(alloc.tensor_shape)
            dtype = mybir.dt.np(alloc.dtype)
            out_avals.append(jax.core.ShapedArray(shape, dtype))
            zero_outs.append(np.zeros(shape, dtype))
    n_params = len(in_names)
    bind_names = tuple(
        in_names + out_names + ([partition_name] if partition_name else [])
    )

    def _body(*args):
        operands = list(args)
        if partition_name is not None:
            operands.append(partition_id_tensor())
        outs = _bass_exec_p.bind(
            *operands,
            out_avals=tuple(out_avals),
            in_names=bind_names,
            out_names=tuple(out_names),
            lowering_input_output_aliases=(),
            sim_require_finite=True,
            sim_require_nnan=True,
            nc=nc,
        )
        return tuple(outs)

    devices = jax.devices()[:n_cores]
    mesh = Mesh(np.asarray(devices), ("core",))
    fn = jax.jit(
        shard_map(
            _body,
            mesh=mesh,
            in_specs=(PartitionSpec("core"),) * (n_params + len(out_names)),
            out_specs=(PartitionSpec("core"),) * len(out_names),
            check_rep=False,
        )
    )
    concat_in = [
        np.concatenate([np.asarray(m[name]) for m in in_maps], axis=0)
        for name in in_names
    ]
    concat_zero = [
        np.zeros((n_cores * z.shape[0], *z.shape[1:]), z.dtype) for z in zero_outs
    ]
    dev_args = [jax.device_put(a) for a in concat_in + concat_zero]
    return fn, dev_args


def bench_exec_ns(cfg, alphas, in_maps, nc1, reps_hi=41, timing_reps=7):
    """Difference wall time of a 1-rep vs reps_hi-rep program (kernel body
    repeated inside one NEFF) to cancel the dispatch round trip."""
    import time

    import jax

    nc_hi = _build_program(cfg, alphas, reps=reps_hi)
    results = {}
    for label, nc in (("lo", nc1), ("hi", nc_hi)):
        fn, dev_args = _make_runner(nc, in_maps)
        jax.block_until_ready(fn(*dev_args))  # compile + warm
        best = float("inf")
        for _ in range(timing_reps):
            t0 = time.perf_counter()
            jax.block_until_ready(fn(*dev_args))
            best = min(best, time.perf_counter() - t0)
        results[label] = best
    exec_ns = (results["hi"] - results["lo"]) / (reps_hi - 1) * 1e9
    return exec_ns, results


# ---------------------------------------------------------------------------
# entry point
# ---------------------------------------------------------------------------

def kernel(h, edge_rows, edge_cols, edge_vals, node_type,
           W_fc, prelu_a, Wg, bg, Wb, bb, film_bias,
           att_W1, att_b1, att_w2, _run_opts=None):
    _ensure_path()
    from concourse import bass_utils

    h = np.asarray(h, dtype=F32)
    edge_rows = np.asarray(edge_rows)
    edge_cols = np.asarray(edge_cols)
    edge_vals = np.asarray(edge_vals, dtype=F32)
    node_type = np.asarray(node_type)

    cfg, per_core = _plan(h, edge_rows, edge_cols, edge_vals, node_type)
    wmats, cvec, cvec16 = _pack_weights(
        cfg, np.asarray(W_fc), np.asarray(prelu_a),
        np.asarray(Wg), np.asarray(bg),
        np.asarray(Wb), np.asarray(bb),
        np.asarray(film_bias), np.asarray(att_W1),
        np.asarray(att_b1), np.asarray(att_w2))

    nc = _build_program(cfg, np.asarray(prelu_a, dtype=F32))

    npc = cfg["npc"]
    B0 = cfg["B0"]
    NCOL = cfg["NCOL"]
    h16 = h.astype(BF16)
    in_maps = []
    for c in range(N_CORES):
        pc = per_core[c]
        hT_own = np.zeros((D, NCOL), dtype=BF16)
        own = h16[c * npc:(c + 1) * npc]      # [npc, D]
        srt = own[pc["perm"]]                 # type-sorted rows
        n0 = pc["n0"]
        hT_own[:, :n0] = srt[:n0].T
        hT_own[:, B0:B0 + (npc - n0)] = srt[n0:].T
        im = {
            "h_tab": h16,
            "hT": hT_own,
            "wmats": wmats,
            "cvec": cvec,
            "cvec16": cvec16,
        }
        for tag in ("L", "H"):
            arr = pc["blob" + tag]
            if arr.shape[1] == 0:  # empty stream: dram tensor padded
                arr = np.full((CHUNK, MCOL), -1, dtype=np.int16)
            im["blob" + tag] = arr
        in_maps.append(im)

    run_kwargs = dict(_run_opts or {})
    bench = run_kwargs.pop("bench", None)
    run_kwargs.pop("_result", None)
    run_kwargs.pop("_bench_ns", None)
    run_kwargs.pop("_bench_times", None)
    res = bass_utils.run_bass_kernel_spmd(
        nc, in_maps, core_ids=list(range(N_CORES)), **run_kwargs
    )

    out = np.empty((cfg["N"], D), dtype=F32)
    for c in range(N_CORES):
        pc = per_core[c]
        n0 = pc["n0"]
        zT = res.results[c]["outT"].astype(F32)   # [D, NCOL] bf16 -> f32
        real = np.concatenate(
            [zT[:, :n0], zT[:, B0:B0 + (npc - n0)]], axis=1
        ).T                                    # [npc, D] sorted order
        shard = np.empty((npc, D), dtype=F32)
        shard[pc["perm"]] = real
        out[c * npc:(c + 1) * npc] = shard + h[c * npc:(c + 1) * npc]

    if bench:
        exec_ns, times = bench_exec_ns(
            cfg, np.asarray(prelu_a, dtype=F32), in_maps, nc)
        if isinstance(_run_opts, dict):
            _run_opts["_bench_ns"] = exec_ns
            _run_opts["_bench_times"] = times
    if isinstance(_run_opts, dict):
        _run_opts["_result"] = res
    return out
